# revision 1
# baseline (speedup 1.0000x reference)
"""Single-head causal attention (B=4, T=2048, C=1024) on 8 trn2 NeuronCores.

Sharding: 8 shards = (batch b in 0..3) x (query interleave h in 0..1).
Query rows are sharded as interleaved 256-row blocks (core h takes global
blocks {2*bg+h}), which balances the causal triangle across the core pair:
every core's four query blocks have causal extents {h, 2+h, 4+h, 6+h}
(x256 kv rows). One SPMD instruction stream serves all cores; all per-core
variation is data: gathered x slices and three [128,128] mask tiles
(m1d/m1f/m2d) that encode whether each kv block is this core's diagonal,
its future, or its past.

Device layout per core (S^T formulation -- scores kept as [kv, query] so
softmax denominators come from ones-matmuls on the TensorE and att@V
needs no transposes):
  phase A: k^T and V for kv global half 0 (kept in SBUF) and half 1
           (spilled to DRAM scratch, contiguous tile-major); q^T last from
           the gathered interleaved rows (reusing the x tile slots).
           DMA emission order is hand-matched to consumption order (the
           sync queue is serial at ~0.65us issue per descriptor).
  phase B: kv half 0 vs all query slots; per-kv-tile valid column ranges
           and mask positions come from static tables (LO128/MASKS);
           row-sums accumulate in PSUM; O^T += V^T A^T over exact ranges.
  phase C: reload half-1 k^T/V from scratch (overlaps B2).
  phase D: kv half 1; only query slots 2,3 participate (cols 512+),
           so this phase is half-sized -- the balance win.
  phase E: normalize by 1/rowsum, output projection with folded bias
           (b_eff = b_proj + w_proj @ b_v), DMA out y^T tile-major.

All matmuls run as float32r (TF32: 4x faster than fp32, max rel err
~4e-4 end-to-end); matmul chunks are kept >=256 wide (f32r is 4x slower
below that). Softmax skips max-subtraction (scores are O(1) here;
mathematically identical). Weights are host-packed into lhsT column-block
layout so every weight load is a single contiguous DMA. The scale 1/sqrt(C)
is folded into W_q/b_q; the V bias into the output bias.
"""

import sys

sys.path.insert(0, "/opt/trn_rl_repo")

import numpy as np

import concourse.bass as bass
import concourse.tile as tile
from concourse import mybir
from concourse.vector_clock import ScopedClock

FP = mybir.dt.float32
FPR = mybir.dt.float32r
AF = mybir.ActivationFunctionType

P = 128
C = 1024  # embed dim
H = 1024  # query rows per core
TL = 2048  # local kv length (own half first, then other half)
NT = C // P  # 8 tiles of 128
NEG = -1.0e9

# toggle: run matmuls as float32r (4x faster PE, slightly different numerics)
USE_F32R = True

_MAX_WAITS = 1


class _TC(tile.TileContext):
    """TileContext whose tail drain puts its global-clock waits on a nop
    (walrus rejects multi-wait Drain); excess waits are split by
    _split_waits() afterwards."""

    def _drain_and_barrier(self, tick_clock, wait_clock):
        nop_inst = self.nc.sync.nop(nofuse=True, hint="pre_drain_waits")
        wait_clock.add_sem_waits(
            nop_inst.ins, ScopedClock({None: tick_clock.global_clock})
        )
        self.nc.sync.drain()
        self.nc.all_engine_barrier()
        assert self.sems is not None
        popped = self.nc._tile_sem_poison_stack.pop()
        assert popped is self._sem_poison
        self.nc.clear_and_free_semaphores(list(self.sems.allocated().values()))
        self.nc.all_engine_barrier()


def _split_waits(nc, max_waits=_MAX_WAITS):
    """The walrus shipped here rejects instructions carrying more than
    `max_waits` sync waits. Move excess waits onto injected nops placed
    immediately before the instruction on the same engine (identical
    semantics: the engine's sequencer blocks on all of them either way)."""
    import copy

    template = nc.sync.nop(nofuse=True, hint="waitsplit_template").ins
    counter = [0]

    def make_nop(engine, waits):
        nop = copy.deepcopy(template)
        counter[0] += 1
        nop.name = f"I-wsplit-{counter[0]}"
        nop.engine = engine
        nop.sync_info = mybir.SyncInfo(on_wait=list(waits), on_update=[])
        return nop

    f = nc.m.functions[0]
    for bb in f.blocks:
        insts = bb.instructions
        if not any(
            i.sync_info and i.sync_info.on_wait and len(i.sync_info.on_wait) > max_waits
            for i in insts
        ):
            continue
        newlist = []
        for inst in insts:
            si = inst.sync_info
            if si and si.on_wait and len(si.on_wait) > max_waits:
                if inst.name == template.name:
                    newlist.append(inst)
                    continue
                waits = list(si.on_wait)
                del si.on_wait[max_waits:]
                rest = waits[max_waits:]
                while rest:
                    newlist.append(make_nop(inst.engine, rest[:max_waits]))
                    rest = rest[max_waits:]
            newlist.append(inst)
        bb.instructions[:] = newlist


MDT = FPR if USE_F32R else FP


def _mm(ap):
    return ap


def _chunks(lo, hi, step=512):
    """Split [lo, hi) into pieces <=step, avoiding <256-wide pieces where
    possible (float32r matmuls run 4x slower below 256 moving rows)."""
    out = []
    while lo < hi:
        rem = hi - lo
        if rem <= step:
            w = rem
        elif rem < step + 256:
            w = rem - 256  # leave a >=256 tail
        else:
            w = step
        out.append((lo, lo + w))
        lo += w
    return out


def _build_nc():
    nc = bass.Bass("TRN2", target_bir_lowering=False, debug=False)

    xTq = nc.dram_tensor("xTq", [C, H], MDT, kind="ExternalInput").ap()
    xTo = nc.dram_tensor("xTo", [C, H], MDT, kind="ExternalInput").ap()
    xTx = nc.dram_tensor("xTx", [C, H], MDT, kind="ExternalInput").ap()
    wqT = nc.dram_tensor("wqT", [C, C], MDT, kind="ExternalInput").ap()
    wkT = nc.dram_tensor("wkT", [C, C], MDT, kind="ExternalInput").ap()
    wvT = nc.dram_tensor("wvT", [4 * P, 4 * 512], MDT, kind="ExternalInput").ap()
    wpT = nc.dram_tensor("wpT", [C, C], MDT, kind="ExternalInput").ap()
    bq = nc.dram_tensor("bq", [P, NT], FP, kind="ExternalInput").ap()
    bk = nc.dram_tensor("bk", [P, NT], FP, kind="ExternalInput").ap()
    beff = nc.dram_tensor("beff", [P, NT], FP, kind="ExternalInput").ap()
    ones_in = nc.dram_tensor("ones_in", [P, P], MDT, kind="ExternalInput").ap()
    m1d_in = nc.dram_tensor("m1d_in", [P, P], FP, kind="ExternalInput").ap()
    m1f_in = nc.dram_tensor("m1f_in", [P, P], FP, kind="ExternalInput").ap()
    m2d_in = nc.dram_tensor("m2d_in", [P, P], FP, kind="ExternalInput").ap()
    # output in (o2-tile, chunk)-major layout; host reassembles
    yT = nc.dram_tensor("yT", [NT * 2 * P, 512], FP, kind="ExternalOutput").ap()
    # DRAM scratch for the other half's kT / V (tile-major, contiguous spills)
    skT = nc.dram_tensor("skT", [2 * C, 512], MDT)
    sV = nc.dram_tensor("sV", [4 * H, 256], MDT)

    with _TC(nc) as tc:
        with (
            tc.tile_pool(name="misc", bufs=1) as misc,
            tc.tile_pool(name="wstream", bufs=3) as wsp,
            tc.tile_pool(name="kqv", bufs=1) as kqv,
            tc.tile_pool(name="evac", bufs=3) as evac,
            tc.tile_pool(name="psum", bufs=6, space="PSUM") as pp,
            tc.tile_pool(name="psum_rs", bufs=1, space="PSUM") as pp_rs,
        ):
            # ---- constants / biases (DMAs emitted after critical loads) --
            ones_sb = misc.tile([P, P], MDT, tag="ones")
            m1d = misc.tile([P, P], FP, tag="m1d")
            m1f = misc.tile([P, P], FP, tag="m1f")
            m2d = misc.tile([P, P], FP, tag="m2d")
            bq_sb = misc.tile([P, NT], FP, tag="bq")
            bk_sb = misc.tile([P, NT], FP, tag="bk")
            beff_sb = misc.tile([P, NT], FP, tag="beff")

            # ---- persistent per-phase tensors ---------------------------
            kT = [kqv.tile([P, H], MDT, tag=f"kT{i}", name=f"kT{i}") for i in range(NT)]
            V = [kqv.tile([P, C], MDT, tag=f"V{i}", name=f"V{i}") for i in range(NT)]
            qT = [kqv.tile([P, H], MDT, tag=f"qT{i}", name=f"qT{i}") for i in range(NT)]

            # =============================================================
            # Phase A: projections (xh holds all of x^T, freed afterwards)
            # =============================================================
            with tc.tile_pool(name="xh", bufs=1) as xp:
                # kv-proj first from global-order x halves; q-proj last from
                # gathered interleaved rows (xq reuses the xho slots)
                xho = [
                    xp.tile([P, H], MDT, tag=f"xho{i}", name=f"xho{i}")
                    for i in range(NT)
                ]
                xhx = [
                    xp.tile([P, H], MDT, tag=f"xhx{i}", name=f"xhx{i}")
                    for i in range(NT)
                ]
                xhalf = [xho, xhx]
                wcol_pre = wsp.tile([P, C], MDT, tag="wcol", name="wcol_pre")
                nc.sync.dma_start(wcol_pre[:], wkT[0:P, :])
                nc.sync.dma_start(bk_sb[:], bk[:])
                for i in range(NT):
                    nc.sync.dma_start(xho[i][:], xTo[i * P : (i + 1) * P, :])

                VW = 256

                # k^T: out tile [o:128, t-chunk], lhsT = w-col slice
                def kproj(half, wv_pre=None):
                    for ot in range(NT):
                        if wv_pre is not None and ot in (4, 6):
                            oc = (ot - 4) // 2
                            t = wsp.tile(
                                [P, NT * VW], MDT, tag="wvoc", bufs=2,
                                name=f"wvp{half}_{oc}",
                            )
                            nc.sync.dma_start(t[:], wvT[oc * P : (oc + 1) * P, :])
                            wv_pre.append(t)
                        osl = slice(ot * P, (ot + 1) * P)
                        if half == 0 and ot == 0:
                            wcol = wcol_pre
                        else:
                            wcol = wsp.tile([P, C], MDT, tag="wcol", name=f"wk{half}_{ot}")
                            nc.sync.dma_start(wcol[:], wkT[osl, :])
                        for (cs, ce) in _chunks(0, H):
                            ps = pp.tile([P, 512], FP, tag="ps", name=f"psk{half}_{ot}_{cs}")
                            w = ce - cs
                            for ct in range(NT):
                                nc.tensor.matmul(
                                    ps[:, :w],
                                    lhsT=_mm(wcol[:, ct * P : (ct + 1) * P]),
                                    rhs=_mm(xhalf[half][ct][:, cs:ce]),
                                    start=(ct == 0),
                                    stop=(ct == NT - 1),
                                )
                            if half == 0:
                                nc.scalar.activation(
                                    kT[ot][:, cs:ce],
                                    ps[:, :w],
                                    AF.Identity,
                                    bias=bk_sb[:, ot : ot + 1],
                                )
                            else:  # spill global-half-1 kT to DRAM
                                ev = evac.tile([P, 512], MDT, tag="ev", name=f"evk{ot}_{cs}")
                                nc.scalar.activation(
                                    ev[:, :w],
                                    ps[:, :w],
                                    AF.Identity,
                                    bias=bk_sb[:, ot : ot + 1],
                                )
                                ci = cs // 512
                                nc.sync.dma_start(
                                    skT[ci * C + ot * P : ci * C + (ot + 1) * P, :w],
                                    ev[:, :w],
                                )

                # V: out tile [t:128, o-chunk], lhsT = xh col slice
                def vproj(half, pre=None):
                    for oc in range(C // VW):
                        ocs = slice(oc * VW, (oc + 1) * VW)
                        if pre is not None and oc < len(pre):
                            wvoc = pre[oc]
                        else:
                            wvoc = wsp.tile(
                                [P, NT * VW], MDT, tag="wvoc", bufs=2,
                                name=f"wv{half}_{oc}",
                            )
                            nc.sync.dma_start(wvoc[:], wvT[oc * P : (oc + 1) * P, :])
                        for tt in range(NT):
                            ps = pp.tile([P, 512], FP, tag="ps", name=f"psv{half}_{oc}_{tt}")
                            tsl = slice(tt * P, (tt + 1) * P)
                            for ct in range(NT):
                                nc.tensor.matmul(
                                    ps[:, :VW],
                                    lhsT=_mm(xhalf[half][ct][:, tsl]),
                                    rhs=_mm(wvoc[:, ct * VW : (ct + 1) * VW]),
                                    start=(ct == 0),
                                    stop=(ct == NT - 1),
                                )
                            if half == 0:
                                nc.vector.tensor_copy(V[tt][:, ocs], ps[:, :VW])
                            else:
                                ev = evac.tile([P, 512], MDT, tag="ev", name=f"evv{oc}_{tt}")
                                nc.vector.tensor_copy(ev[:, :VW], ps[:, :VW])
                                nc.sync.dma_start(
                                    sV[oc * H + tt * P : oc * H + (tt + 1) * P, :VW],
                                    ev[:, :VW],
                                )

                sc_qk = tc.nc.named_scope("A_qk")
                sc_qk.__enter__()
                kproj(0)
                sc_qk.__exit__(None, None, None)
                for i in range(NT):
                    nc.sync.dma_start(xhx[i][:], xTx[i * P : (i + 1) * P, :])
                nc.sync.dma_start(ones_sb[:], ones_in[:])
                nc.sync.dma_start(m1d[:], m1d_in[:])
                nc.sync.dma_start(m1f[:], m1f_in[:])
                nc.sync.dma_start(m2d[:], m2d_in[:])
                nc.sync.dma_start(bq_sb[:], bq[:])
                nc.sync.dma_start(beff_sb[:], beff[:])
                sc_v = tc.nc.named_scope("A_v")
                sc_v.__enter__()
                vproj(0)
                sc_v.__exit__(None, None, None)
                sc_qk = tc.nc.named_scope("A_qk2")
                sc_qk.__enter__()
                wv_pre = []
                kproj(1, wv_pre)
                sc_qk.__exit__(None, None, None)
                sc_v = tc.nc.named_scope("A_v2")
                sc_v.__enter__()
                vproj(1, wv_pre)
                sc_v.__exit__(None, None, None)

                # q^T last: xq tiles reuse the xho slots (WAR-ordered)
                sc_q = tc.nc.named_scope("A_q")
                sc_q.__enter__()
                xq = [
                    xp.tile([P, H], MDT, tag=f"xho{i}", name=f"xq{i}")
                    for i in range(NT)
                ]
                for i in range(NT):
                    nc.sync.dma_start(xq[i][:], xTq[i * P : (i + 1) * P, :])
                for ot in range(NT):
                    osl = slice(ot * P, (ot + 1) * P)
                    wcol = wsp.tile([P, C], MDT, tag="wcol")
                    nc.sync.dma_start(wcol[:], wqT[osl, :])
                    for (cs, ce) in _chunks(0, H):
                        ps = pp.tile([P, 512], FP, tag="ps")
                        w = ce - cs
                        for ct in range(NT):
                            nc.tensor.matmul(
                                ps[:, :w],
                                lhsT=_mm(wcol[:, ct * P : (ct + 1) * P]),
                                rhs=_mm(xq[ct][:, cs:ce]),
                                start=(ct == 0),
                                stop=(ct == NT - 1),
                            )
                        nc.scalar.activation(
                            qT[ot][:, cs:ce],
                            ps[:, :w],
                            AF.Identity,
                            bias=bq_sb[:, ot : ot + 1],
                        )
                sc_q.__exit__(None, None, None)

            # =============================================================
            # Phases B-E (attention): xh freed, AT/Oacc reuse its space
            # =============================================================
            with tc.tile_pool(name="attn", bufs=1) as ab:
                AT = [ab.tile([P, H], MDT, tag=f"AT{i}", name=f"AT{i}") for i in range(NT)]
                Oacc = [ab.tile([P, H], MDT, tag=f"O{i}", name=f"O{i}") for i in range(NT)]
                rs_sb = ab.tile([P, H], FP, tag="rs_sb")
                rs_ps = pp_rs.tile([P, H], FP, tag="rs")

                # Interleaved-256 balanced causal structure.
                # Query slots bg=0..3 hold global 256-row blocks g=2*bg+h.
                # Per kv 128-tile s (within a 512-col kv phase):
                #   valid query cols [LO128[s]*128 + 512*p, 1024)
                #   mask adds (m1d/m1f/m2d data tiles) at fixed positions.
                LO128 = [0, 0, 0, 1, 2, 2, 2, 3]
                MASKS = [
                    [(0, "m1d")],
                    [(0, "m1f"), (1, "m1d")],
                    [(0, "m2d"), (1, "m1f")],
                    [(1, "m2d")],
                    [(2, "m1d")],
                    [(2, "m1f"), (3, "m1d")],
                    [(2, "m2d"), (3, "m1f")],
                    [(3, "m2d")],
                ]
                MT = {"m1d": m1d, "m1f": m1f, "m2d": m2d}

                def scores_phase(pphase, first_rs, last_rs):
                    base = 512 * pphase
                    for s in range(NT):
                        lo = base + LO128[s] * P
                        chs = _chunks(lo, H)
                        # ct-outer: the chunk pair shares each kT lhsT, so the
                        # PE loads each stationary operand once, not twice
                        pss = [
                            pp.tile([P, 512], FP, tag="ps", name=f"pss{pphase}_{s}_{i}")
                            for i in range(len(chs))
                        ]
                        for ct in range(NT):
                            lhsT = _mm(kT[ct][:, s * P : (s + 1) * P])
                            for ps, (cs, ce) in zip(pss, chs):
                                nc.tensor.matmul(
                                    ps[:, : ce - cs],
                                    lhsT=lhsT,
                                    rhs=_mm(qT[ct][:, cs:ce]),
                                    start=(ct == 0),
                                    stop=(ct == NT - 1),
                                )
                        for ps, (cs, ce) in zip(pss, chs):
                            w = ce - cs
                            for off, mname in MASKS[s]:
                                a = base + off * P
                                if cs <= a < ce:
                                    nc.vector.tensor_add(
                                        ps[:, a - cs : a - cs + P],
                                        ps[:, a - cs : a - cs + P],
                                        MT[mname][:],
                                    )
                            nc.scalar.activation(AT[s][:, cs:ce], ps[:, :w], AF.Exp)
                    for s in range(NT):
                        lo = base + LO128[s] * P
                        for (cs, ce) in _chunks(lo, H):
                            nc.tensor.matmul(
                                rs_ps[:, cs:ce],
                                lhsT=_mm(ones_sb[:]),
                                rhs=_mm(AT[s][:, cs:ce]),
                                start=(first_rs and s == 0),
                                stop=(last_rs and s == NT - 1),
                            )

                def attv_phase(pphase, accumulate):
                    base = 512 * pphase
                    chs = _chunks(base, H)
                    for ot in range(NT):
                        osl = slice(ot * P, (ot + 1) * P)
                        # s-inner with one psum per chunk: each V lhsT loads once
                        pss = [
                            pp.tile([P, 512], FP, tag="ps", name=f"psav{pphase}_{ot}_{i}")
                            for i in range(len(chs))
                        ]
                        for s in range(NT):
                            lhsT = _mm(V[s][:, osl])
                            for ps, (cs, ce) in zip(pss, chs):
                                lo = max(cs, base + LO128[s] * P)
                                if lo >= ce:
                                    continue
                                smax = min(ce // P, NT)
                                nc.tensor.matmul(
                                    ps[:, lo - cs : ce - cs],
                                    lhsT=lhsT,
                                    rhs=_mm(AT[s][:, lo:ce]),
                                    start=(s == 0),
                                    stop=(s == NT - 1),
                                )
                        for ps, (cs, ce) in zip(pss, chs):
                            if accumulate:
                                nc.vector.tensor_add(
                                    Oacc[ot][:, cs:ce], Oacc[ot][:, cs:ce], ps[:]
                                )
                            else:
                                nc.vector.tensor_copy(Oacc[ot][:, cs:ce], ps[:])

                sc = tc.nc.named_scope("B1"); sc.__enter__()
                scores_phase(0, True, False)
                sc.__exit__(None, None, None)
                sc = tc.nc.named_scope("B2"); sc.__enter__()
                attv_phase(0, False)
                sc.__exit__(None, None, None)

                sc = tc.nc.named_scope("C"); sc.__enter__()
                # ---- phase C: reload kv global half 1 -------------------
                for i in range(NT):
                    for ci in range(2):
                        nc.sync.dma_start(
                            kT[i][:, ci * 512 : (ci + 1) * 512],
                            skT[ci * C + i * P : ci * C + (i + 1) * P, :],
                        )
                    for oc in range(4):
                        nc.sync.dma_start(
                            V[i][:, oc * 256 : (oc + 1) * 256],
                            sV[oc * H + i * P : oc * H + (i + 1) * P, :],
                        )
                sc.__exit__(None, None, None)

                sc = tc.nc.named_scope("D1"); sc.__enter__()
                scores_phase(1, False, True)
                sc.__exit__(None, None, None)
                sc = tc.nc.named_scope("D2"); sc.__enter__()
                attv_phase(1, True)
                sc.__exit__(None, None, None)

                # Oacc cols [0:512) got no phase-D contribution by
                # construction (query slots 0,1 never see kv half 1).

                sc = tc.nc.named_scope("E"); sc.__enter__()
                # ---- phase E: normalize + output projection -------------
                nc.vector.tensor_copy(rs_sb[:], rs_ps[:])
                nc.vector.reciprocal(rs_sb[:], rs_sb[:])
                for ot in range(NT):
                    nc.vector.tensor_mul(Oacc[ot][:], Oacc[ot][:], rs_sb[:])

                for o2 in range(NT):
                    osl = slice(o2 * P, (o2 + 1) * P)
                    wcol = wsp.tile([P, C], MDT, tag="wcol")
                    nc.sync.dma_start(wcol[:], wpT[osl, :])
                    echs = _chunks(0, H)
                    pss = [
                        pp.tile([P, 512], FP, tag="ps", name=f"pse{o2}_{i}")
                        for i in range(len(echs))
                    ]
                    for ot in range(NT):
                        lhsT = _mm(wcol[:, ot * P : (ot + 1) * P])
                        for ps, (cs, ce) in zip(pss, echs):
                            nc.tensor.matmul(
                                ps[:],
                                lhsT=lhsT,
                                rhs=_mm(Oacc[ot][:, cs:ce]),
                                start=(ot == 0),
                                stop=(ot == NT - 1),
                            )
                    for ps, (cs, ce) in zip(pss, echs):
                        ev = evac.tile([P, 512], FP, tag="evy")
                        nc.scalar.activation(
                            ev[:], ps[:], AF.Identity, bias=beff_sb[:, o2 : o2 + 1]
                        )
                        ci = cs // 512
                        nc.sync.dma_start(
                            yT[(o2 * 2 + ci) * P : (o2 * 2 + ci + 1) * P, :], ev[:]
                        )
                sc.__exit__(None, None, None)

    _split_waits(nc)
    return nc


_NC_CACHE = None


def _get_nc():
    global _NC_CACHE
    if _NC_CACHE is None:
        _NC_CACHE = _build_nc()
    return _NC_CACHE


def make_in_maps(x, w_qkv, b_qkv, w_proj, b_proj):
    """Host-side prep: shard + transpose inputs for the 8 cores."""
    x = np.asarray(x, dtype=np.float32)
    w_qkv = np.asarray(w_qkv, dtype=np.float32)
    b_qkv = np.asarray(b_qkv, dtype=np.float32)
    w_proj = np.asarray(w_proj, dtype=np.float32)
    b_proj = np.asarray(b_proj, dtype=np.float32)

    s = 1.0 / np.sqrt(np.float32(C))

    def pack_cols(w, bw=P):
        # [ot*bw + p(in-part), ct*P + o(out-within)] = w[ot*bw + o, ct*P + p]
        n_o = C // bw
        w4 = w.reshape(n_o, bw, NT, P).transpose(0, 3, 2, 1)
        return np.ascontiguousarray(w4).reshape(n_o * P, NT * bw)

    wqT = pack_cols(w_qkv[0:C] * s)
    wkT = pack_cols(w_qkv[C : 2 * C])
    wvT = pack_cols(w_qkv[2 * C : 3 * C], bw=256)
    wpT = pack_cols(w_proj)
    bq = np.ascontiguousarray((b_qkv[0:C] * s).reshape(NT, P).T)
    bk = np.ascontiguousarray(b_qkv[C : 2 * C].reshape(NT, P).T)
    bv = b_qkv[2 * C : 3 * C]
    beff = np.ascontiguousarray((b_proj + w_proj @ bv).reshape(NT, P).T)

    ones = np.ones((P, P), dtype=np.float32)
    # S^T layout: partition = kv index j, free = query index i;
    # visible (mask 0) where i >= j within a diagonal 128-block
    triu = np.triu(np.ones((P, P), dtype=np.float32))
    trilm = np.where(triu > 0, 0.0, NEG).astype(np.float32)
    zeros = np.zeros((P, P), dtype=np.float32)
    negs = np.full((P, P), NEG, dtype=np.float32)

    shared = dict(
        wqT=wqT, wkT=wkT, wvT=wvT, wpT=wpT, bq=bq, bk=bk, beff=beff,
        ones_in=ones,
    )
    in_maps = []
    for core in range(8):
        b, h = core // 2, core % 2
        xb = x[b]  # [T, C]
        # query rows: interleaved 256-blocks g = 2*bg + h
        qrows = np.concatenate(
            [xb[(2 * bg + h) * 256 : (2 * bg + h + 1) * 256] for bg in range(4)],
            axis=0,
        )
        in_maps.append(
            dict(
                shared,
                xTq=np.ascontiguousarray(qrows.T),
                xTo=np.ascontiguousarray(xb[0:H].T),
                xTx=np.ascontiguousarray(xb[H : 2 * H].T),
                # block-type masks (see device LO128/MASKS tables):
                # m1d: diagonal of a "diag(h=0)/full(h=1)" block
                # m1f: future-subtile of such a block (or past of T2)
                # m2d: diagonal of a "masked(h=0)/diag(h=1)" block
                m1d_in=trilm if h == 0 else zeros,
                m1f_in=negs if h == 0 else zeros,
                m2d_in=negs if h == 0 else trilm,
            )
        )
    return in_maps


def assemble_output(results):
    B = 4
    y = np.empty((B, 2 * H, C), dtype=np.float32)
    for core in range(8):
        b, h = core // 2, core % 2
        # yT layout [o2, ci, p, 512] -> rows are slot-major query cols
        yt = results[core]["yT"].reshape(NT, 2, P, 512)
        blk = yt.transpose(1, 3, 0, 2).reshape(H, C)  # [slot-major rows, C]
        blk4 = blk.reshape(4, 256, C)
        for bg in range(4):
            g = 2 * bg + h
            y[b, g * 256 : (g + 1) * 256, :] = blk4[bg]
    return y


def kernel(x, w_qkv, b_qkv, w_proj, b_proj):
    from concourse.bass_utils import run_bass_kernel_spmd

    nc = _get_nc()
    in_maps = make_in_maps(x, w_qkv, b_qkv, w_proj, b_proj)
    res = run_bass_kernel_spmd(nc, in_maps, list(range(8)))
    return assemble_output(res.results)



# revision 4
# speedup vs baseline: 1.1340x; 1.1340x over previous
"""Single-head causal attention (B=4, T=2048, C=1024) on 8 trn2 NeuronCores.

Sharding: 8 shards = (batch b in 0..3) x (query interleave h in 0..1), same
balanced interleaved-256 query split as the fp32r baseline: core h of a pair
takes global 256-row query blocks {2*bg+h}, so the causal triangle is
balanced across the pair. One SPMD instruction stream; all per-core
variation is data (gathered x slices + three [128,128] mask tiles).

All matmuls run as compensated fp8e4m3 DoubleRow pairs. Each operand is
decomposed as v = hi + lo (hi = fp8(v), lo = fp8(v - hi)); a product
x*w = xh*wh + (xh*wl + xl*wh) keeps ~bf16 accuracy (residual ~0.1%) while
DoubleRow processes TWO 128-deep contraction planes per instruction at 0.5
cycles/row -- 4x the fp32r/bf16 rate, so the compensated triple costs 0.75x
of the bf16-equivalent. Host-side tensors (x, weights) are decomposed on
the host; device-computed tensors (q, k, v, exp-scores, attention out) are
decomposed with a cast + subtract pass (Act/DVE/Pool engines, all far under
the PE roofline).

Algebraic folds:
  - k bias dropped entirely: softmax over kv positions is invariant to
    per-query constants, and (q+bq).(k+bk) - (q+bq).k is constant per row.
  - q bias folded into the exp: s_ij = q~_i.k_j + (bq~.k_j), the second
    term is per-kv-position, computed on device as a tiny N=1 DoubleRow
    matmul chain (bqk), and applied as the Exp activation bias together
    with -ln(32).
  - exp(s)/32 stored instead of exp(s) so fp8's 240 max is never hit
    (scores ~N(0,1); the 1/32 cancels between att@V and the rowsum).
  - v bias folded into the output bias (beff = b_proj + w_proj @ b_v).
  - 1/sqrt(C) folded into wq/bq host-side.

fp8 halves SBUF, so unlike the baseline there is NO DRAM spill of kv half 1
(phases C/skT/sV are gone): kT/V for all 2048 kv positions, qT, AT and the
fp32 O accumulator are all resident.

Comp-plane storage convention (so compensated cross terms pair cleanly):
"moving-side" tensors (x, qT, AT, Opair, bq) store (hi, lo); "stationary
side" (wk, wq, wv, wp, kT, V) store (lo, hi). A cross op then reads
lhsT[:, 0:2] x rhs[:, 0:2] = wl*xh + wh*xl directly; hi*hi ops index plane
1 of the stationary and plane 0 of the moving tensor, pairing adjacent
contraction subtiles instead.
"""

import sys

sys.path.insert(0, "/opt/trn_rl_repo")

import numpy as np

import concourse.bass as bass
import concourse.tile as tile
from concourse import mybir
from concourse.vector_clock import ScopedClock

FP = mybir.dt.float32
BF = mybir.dt.bfloat16
F8 = mybir.dt.float8e4
AF = mybir.ActivationFunctionType
DR = mybir.MatmulPerfMode.DoubleRow

P = 128
C = 1024  # embed dim
H = 1024  # query rows per core
TKV = 2048  # kv length
NT = C // P  # 8 c-subtiles
NKV = TKV // P  # 16 kv-subtiles
NEG = -1.0e9
ASCALE = 32.0  # exp(s)/ASCALE stored in fp8

_MAX_WAITS = 1


class _TC(tile.TileContext):
    """TileContext whose tail drain puts its global-clock waits on a nop
    (walrus rejects multi-wait Drain); excess waits are split by
    _split_waits() afterwards."""

    def _drain_and_barrier(self, tick_clock, wait_clock):
        nop_inst = self.nc.sync.nop(nofuse=True, hint="pre_drain_waits")
        wait_clock.add_sem_waits(
            nop_inst.ins, ScopedClock({None: tick_clock.global_clock})
        )
        self.nc.sync.drain()
        self.nc.all_engine_barrier()
        assert self.sems is not None
        popped = self.nc._tile_sem_poison_stack.pop()
        assert popped is self._sem_poison
        self.nc.clear_and_free_semaphores(list(self.sems.allocated().values()))
        self.nc.all_engine_barrier()


def _split_waits(nc, max_waits=_MAX_WAITS):
    """The walrus shipped here rejects instructions carrying more than
    `max_waits` sync waits. Move excess waits onto injected nops placed
    immediately before the instruction on the same engine."""
    import copy

    template = nc.sync.nop(nofuse=True, hint="waitsplit_template").ins
    counter = [0]

    def make_nop(engine, waits):
        nop = copy.deepcopy(template)
        counter[0] += 1
        nop.name = f"I-wsplit-{counter[0]}"
        nop.engine = engine
        nop.sync_info = mybir.SyncInfo(on_wait=list(waits), on_update=[])
        return nop

    f = nc.m.functions[0]
    for bb in f.blocks:
        insts = bb.instructions
        if not any(
            i.sync_info and i.sync_info.on_wait and len(i.sync_info.on_wait) > max_waits
            for i in insts
        ):
            continue
        newlist = []
        for inst in insts:
            si = inst.sync_info
            if si and si.on_wait and len(si.on_wait) > max_waits:
                if inst.name == template.name:
                    newlist.append(inst)
                    continue
                waits = list(si.on_wait)
                del si.on_wait[max_waits:]
                rest = waits[max_waits:]
                while rest:
                    newlist.append(make_nop(inst.engine, rest[:max_waits]))
                    rest = rest[max_waits:]
            newlist.append(inst)
        bb.instructions[:] = newlist


# Causal structure for the interleaved-256 query sharding, over 16 kv
# 128-subtiles. Query slots bg=0..3 hold global 256-row blocks g=2*bg+h.
# For kv subtile s, valid query cols start at LO16[s]*128; mask tiles
# (data-encoded per core) are added at the listed 128-col block positions.
LO16 = [0, 0, 0, 1, 2, 2, 2, 3, 4, 4, 4, 5, 6, 6, 6, 7]
_MASKS8 = [
    [(0, 0)],            # (128-block, mask index) ; 0=m1d 1=m1f 2=m2d
    [(0, 1), (1, 0)],
    [(0, 2), (1, 1)],
    [(1, 2)],
    [(2, 0)],
    [(2, 1), (3, 0)],
    [(2, 2), (3, 1)],
    [(3, 2)],
]
MASKS16 = [
    [((s // 8) * 4 + off, mi) for off, mi in _MASKS8[s % 8]] for s in range(16)
]
# pair-aligned lo (attv pairs kv subtiles (2p, 2p+1))
LOP16 = [LO16[s] - (LO16[s] % 2) for s in range(16)]


def _chunks512(lo, hi):
    """Split [lo, hi) at absolute multiples of 512."""
    out = []
    while lo < hi:
        ce = min((lo // 512 + 1) * 512, hi)
        out.append((lo, ce))
        lo = ce
    return out


def _build_nc():
    nc = bass.Bass("TRN2", target_bir_lowering=False, debug=False)

    xq_in = nc.dram_tensor("xq_in", [P, 2, NT, H], F8, kind="ExternalInput").ap()
    xo_in = nc.dram_tensor("xo_in", [P, 2, NT, H], F8, kind="ExternalInput").ap()
    xx_in = nc.dram_tensor("xx_in", [P, 2, NT, H], F8, kind="ExternalInput").ap()
    wk_in = nc.dram_tensor("wk_in", [P, 2, NT, C], F8, kind="ExternalInput").ap()
    wq_in = nc.dram_tensor("wq_in", [P, 2, NT, C], F8, kind="ExternalInput").ap()
    wv_in = nc.dram_tensor("wv_in", [P, 2, NT, C], F8, kind="ExternalInput").ap()
    wp_in = nc.dram_tensor("wp_in", [P, 2, NT, C], F8, kind="ExternalInput").ap()
    bqp_in = nc.dram_tensor("bqp_in", [P, 2, NT, 1], F8, kind="ExternalInput").ap()
    ones_in = nc.dram_tensor("ones_in", [P, 2, P], F8, kind="ExternalInput").ap()
    masks_in = nc.dram_tensor("masks_in", [P, 3, P], FP, kind="ExternalInput").ap()
    # beff (8 cols) | -ln(ASCALE) (1 col)
    bias_in = nc.dram_tensor("bias_in", [P, NT + 1], FP, kind="ExternalInput").ap()
    # output, (o2-tile, chunk)-major, bf16; host reassembles + upcasts
    yT = nc.dram_tensor("yT", [NT * 2 * P, 512], BF, kind="ExternalOutput").ap()

    with _TC(nc) as tc:
        with (
            tc.tile_pool(name="misc", bufs=1) as misc,
            tc.tile_pool(name="kqv", bufs=1) as kqv,
            tc.tile_pool(name="psum", bufs=4, space="PSUM") as pp,
            tc.tile_pool(name="psum_rs", bufs=1, space="PSUM") as pp_rs,
        ):
            ones_sb = misc.tile([P, 2, P], F8, tag="ones")
            masks = misc.tile([P, 3, P], FP, tag="masks")
            bias_sb = misc.tile([P, NT + 1], FP, tag="bias")
            bqp = misc.tile([P, 2, NT, 1], F8, tag="bqp")
            bqk_sb = misc.tile([P, NKV], FP, tag="bqk")

            # persistent fp8 pair tensors (comp order noted)
            kT = kqv.tile([P, 2, NT, TKV], F8, tag="kT")   # (lo, hi)
            qT = kqv.tile([P, 2, NT, H], F8, tag="qT")     # (hi, lo)
            V = kqv.tile([P, 2, NKV, C], F8, tag="V")      # (lo, hi)
            wp = kqv.tile([P, 2, NT, C], F8, tag="wp")     # (lo, hi)

            # =============================================================
            # Phase A: projections
            # =============================================================
            with tc.tile_pool(name="xw", bufs=1) as xw:
                xo = xw.tile([P, 2, NT, H], F8, tag="xo")
                xx = xw.tile([P, 2, NT, H], F8, tag="xx")
                xq = xw.tile([P, 2, NT, H], F8, tag="xq")
                wk = xw.tile([P, 2, NT, C], F8, tag="wk")
                wq = xw.tile([P, 2, NT, C], F8, tag="wq")
                wv = xw.tile([P, 2, NT, C], F8, tag="wv")

                nc.sync.dma_start(xo[:], xo_in[:])
                nc.sync.dma_start(wk[:], wk_in[:])
                nc.sync.dma_start(xx[:], xx_in[:])
                nc.sync.dma_start(xq[:], xq_in[:])
                nc.sync.dma_start(wq[:], wq_in[:])
                nc.sync.dma_start(wv[:], wv_in[:])
                nc.sync.dma_start(ones_sb[:], ones_in[:])
                nc.sync.dma_start(masks[:], masks_in[:])
                nc.sync.dma_start(bias_sb[:], bias_in[:])
                nc.sync.dma_start(bqp[:], bqp_in[:])
                nc.sync.dma_start(wp[:], wp_in[:])

                xhalf = [xo, xx]

                def mm12(ps, w, x, osl, cs, ce, n_start=True, n_stop=True):
                    """12-op compensated group: out[osl, cs:ce] += w.T @ x.
                    w stored (lo,hi), x stored (hi,lo); contraction over all
                    NT c-subtiles."""
                    first = [n_start]
                    for t in range(NT // 2):
                        nc.tensor.matmul(
                            ps[:, : ce - cs],
                            lhsT=w[:, 1, 2 * t : 2 * t + 2, osl],
                            rhs=x[:, 0, 2 * t : 2 * t + 2, cs:ce],
                            start=first[0],
                            stop=False,
                            perf_mode=DR,
                        )
                        first[0] = False
                    for ct in range(NT):
                        nc.tensor.matmul(
                            ps[:, : ce - cs],
                            lhsT=w[:, 0:2, ct, osl],
                            rhs=x[:, 0:2, ct, cs:ce],
                            start=False,
                            stop=(n_stop and ct == NT - 1),
                            perf_mode=DR,
                        )

                # ---- k projection (no bias; softmax-invariant) ----------
                sc = tc.nc.named_scope("A_k"); sc.__enter__()
                for half in range(2):
                    xh = xhalf[half]
                    for ot in range(NT):
                        osl = slice(ot * P, (ot + 1) * P)
                        for cs, ce in ((0, 512), (512, 1024)):
                            ps = pp.tile([P, 512], FP, tag="ps")
                            mm12(ps, wk, xh, osl, cs, ce)
                            g0 = half * H + cs
                            nc.scalar.activation(
                                kT[:, 1, ot, g0 : g0 + 512], ps[:], AF.Identity
                            )
                            nc.vector.tensor_sub(
                                kT[:, 0, ot, g0 : g0 + 512],
                                ps[:],
                                kT[:, 1, ot, g0 : g0 + 512],
                            )
                sc.__exit__(None, None, None)

                # ---- bqk: per-kv-position q-bias term (bq~ . k_j) -------
                sc = tc.nc.named_scope("A_bqk"); sc.__enter__()
                ps_b = pp.tile([P, NKV], FP, tag="psb", bufs=1)
                nop = 0
                for kvt in range(NKV):
                    ksl = slice(kvt * P, (kvt + 1) * P)
                    for t in range(NT // 2):
                        nc.tensor.matmul(
                            ps_b[:, kvt : kvt + 1],
                            lhsT=kT[:, 1, 2 * t : 2 * t + 2, ksl],
                            rhs=bqp[:, 0, 2 * t : 2 * t + 2, :],
                            start=(nop == 0),
                            stop=False,
                            perf_mode=DR,
                            skip_group_check=True,
                        )
                        nop += 1
                    for ct in range(NT):
                        nop += 1
                        nc.tensor.matmul(
                            ps_b[:, kvt : kvt + 1],
                            lhsT=kT[:, 0:2, ct, ksl],
                            rhs=bqp[:, 0:2, ct, :],
                            start=False,
                            stop=(nop == 12 * NKV),
                            perf_mode=DR,
                            skip_group_check=True,
                        )
                # bqk_sb = bqk - ln(ASCALE): the Exp bias for each kv row
                nc.scalar.activation(
                    bqk_sb[:], ps_b[:], AF.Identity, bias=bias_sb[:, NT : NT + 1]
                )
                sc.__exit__(None, None, None)

                # ---- v projection (x stationary, w moving; no bias) -----
                sc = tc.nc.named_scope("A_v"); sc.__enter__()
                for half in range(2):
                    xh = xhalf[half]
                    for tt in range(NT):
                        ts_g = half * NT + tt
                        tsl = slice(tt * P, (tt + 1) * P)
                        for cs, ce in ((0, 512), (512, 1024)):
                            ps = pp.tile([P, 512], FP, tag="ps")
                            first = True
                            for t in range(NT // 2):
                                nc.tensor.matmul(
                                    ps[:],
                                    lhsT=xh[:, 0, 2 * t : 2 * t + 2, tsl],
                                    rhs=wv[:, 1, 2 * t : 2 * t + 2, cs:ce],
                                    start=first,
                                    stop=False,
                                    perf_mode=DR,
                                )
                                first = False
                            for ct in range(NT):
                                nc.tensor.matmul(
                                    ps[:],
                                    lhsT=xh[:, 0:2, ct, tsl],
                                    rhs=wv[:, 0:2, ct, cs:ce],
                                    start=False,
                                    stop=(ct == NT - 1),
                                    perf_mode=DR,
                                )
                            nc.scalar.activation(
                                V[:, 1, ts_g, cs:ce], ps[:], AF.Identity
                            )
                            nc.vector.tensor_sub(
                                V[:, 0, ts_g, cs:ce], ps[:], V[:, 1, ts_g, cs:ce]
                            )
                sc.__exit__(None, None, None)

                # ---- q projection (scaled wq; bias via bqk) -------------
                sc = tc.nc.named_scope("A_q"); sc.__enter__()
                for ot in range(NT):
                    osl = slice(ot * P, (ot + 1) * P)
                    for cs, ce in ((0, 512), (512, 1024)):
                        ps = pp.tile([P, 512], FP, tag="ps")
                        mm12(ps, wq, xq, osl, cs, ce)
                        nc.scalar.activation(
                            qT[:, 0, ot, cs:ce], ps[:], AF.Identity
                        )
                        nc.vector.tensor_sub(
                            qT[:, 1, ot, cs:ce], ps[:], qT[:, 0, ot, cs:ce]
                        )
                sc.__exit__(None, None, None)

            # =============================================================
            # Phases B-D (attention): xw freed; AT/Oacc/Opair reuse space
            # =============================================================
            with (
                tc.tile_pool(name="attn", bufs=1) as ab,
                tc.tile_pool(name="efp", bufs=3) as efp,
            ):
                AT = ab.tile([P, 2, NKV, H], F8, tag="AT")   # (hi, lo)
                Oacc = ab.tile([P, NT, H], FP, tag="Oacc")
                Opair = ab.tile([P, 2, NT, H], F8, tag="Op")  # (hi, lo)
                rs_sb = ab.tile([P, H], FP, tag="rs_sb")
                rs_ps = pp_rs.tile([P, H], FP, tag="rs")

                ef_cur = [None]

                def scores_s(s):
                    lo_s = LO16[s] * P
                    lo_p = LOP16[s] * P
                    if s % 2 == 0:
                        ef_cur[0] = efp.tile([P, 2, H], BF, tag="ef", name=f"ef{s}")
                    ef = ef_cur[0]
                    chs = _chunks512(lo_p, H)
                    pss = [
                        pp.tile([P, ce - cs], FP, tag="ps", name=f"pss{s}_{cs}")
                        for cs, ce in chs
                    ]
                    # ct-outer so each stationary kT slice loads once
                    nop, last = 0, 12 * len(chs)
                    for t in range(NT // 2):
                        for ps, (cs, ce) in zip(pss, chs):
                            mlo = max(cs, lo_s)
                            nc.tensor.matmul(
                                ps[:, mlo - cs : ce - cs],
                                lhsT=kT[:, 1, 2 * t : 2 * t + 2, s * P : (s + 1) * P],
                                rhs=qT[:, 0, 2 * t : 2 * t + 2, mlo:ce],
                                start=(nop < len(chs)),
                                stop=False,
                                perf_mode=DR,
                                skip_group_check=True,
                            )
                            nop += 1
                    for ct in range(NT):
                        for ps, (cs, ce) in zip(pss, chs):
                            mlo = max(cs, lo_s)
                            nop += 1
                            nc.tensor.matmul(
                                ps[:, mlo - cs : ce - cs],
                                lhsT=kT[:, 0:2, ct, s * P : (s + 1) * P],
                                rhs=qT[:, 0:2, ct, mlo:ce],
                                start=False,
                                stop=(nop > last - len(chs)),
                                perf_mode=DR,
                                skip_group_check=True,
                            )
                    # dead sliver [lo_p, lo_s): exp(-1e9) = 0 keeps the fp8
                    # pair exactly zero there so paired attv ops read zeros
                    if lo_s > lo_p:
                        nc.vector.memset(pss[0][:, 0 : lo_s - lo_p], NEG)
                    for ps, (cs, ce) in zip(pss, chs):
                        for blk, mi in MASKS16[s]:
                            a = blk * P
                            if cs <= a < ce:
                                nc.vector.tensor_add(
                                    ps[:, a - cs : a - cs + P],
                                    ps[:, a - cs : a - cs + P],
                                    masks[:, mi, :],
                                )
                        nc.scalar.activation(
                            ef[:, s % 2, cs:ce],
                            ps[:],
                            AF.Exp,
                            bias=bqk_sb[:, s : s + 1],
                        )
                    if s % 2 == 1:
                        # pair complete: decompose exp into the AT fp8 pair
                        nc.gpsimd.tensor_copy(
                            AT[:, 0, s - 1 : s + 1, lo_p:H], ef[:, :, lo_p:H]
                        )
                        nc.gpsimd.tensor_sub(
                            AT[:, 1, s - 1 : s + 1, lo_p:H],
                            ef[:, :, lo_p:H],
                            AT[:, 0, s - 1 : s + 1, lo_p:H],
                        )
                        # rowsum for both subtiles of the pair
                        for sj in (s - 1, s):
                            lo_j = LOP16[sj] * P
                            for cs, ce in _chunks512(lo_j, H):
                                nc.tensor.matmul(
                                    rs_ps[:, cs:ce],
                                    lhsT=ones_sb[:],
                                    rhs=AT[:, 0:2, sj, cs:ce],
                                    start=(sj == 0),
                                    stop=(
                                        (cs < 512 and sj == 7)
                                        or (cs >= 512 and sj == NKV - 1)
                                    ),
                                    perf_mode=DR,
                                    skip_group_check=True,
                                )

                def attv_chunk(ci, pairs):
                    cs, ce = ci * 512, (ci + 1) * 512
                    for ot in range(NT):
                        osl = slice(ot * P, (ot + 1) * P)
                        ps = pp.tile([P, 512], FP, tag="ps")
                        nops = 3 * len(pairs)
                        nop = 0
                        for p in pairs:
                            plo = max(cs, LOP16[2 * p] * P)
                            nc.tensor.matmul(
                                ps[:, plo - cs : 512],
                                lhsT=V[:, 1, 2 * p : 2 * p + 2, osl],
                                rhs=AT[:, 0, 2 * p : 2 * p + 2, plo:ce],
                                start=(nop == 0),
                                stop=False,
                                perf_mode=DR,
                                skip_group_check=True,
                            )
                            nop += 1
                            for sj in (2 * p, 2 * p + 1):
                                nop += 1
                                nc.tensor.matmul(
                                    ps[:, plo - cs : 512],
                                    lhsT=V[:, 0:2, sj, osl],
                                    rhs=AT[:, 0:2, sj, plo:ce],
                                    start=False,
                                    stop=(nop == nops),
                                    perf_mode=DR,
                                    skip_group_check=True,
                                )
                        nc.vector.tensor_copy(Oacc[:, ot, cs:ce], ps[:])

                sc = tc.nc.named_scope("B1"); sc.__enter__()
                for s in range(8):
                    scores_s(s)
                sc.__exit__(None, None, None)
                sc = tc.nc.named_scope("B2"); sc.__enter__()
                attv_chunk(0, [0, 1, 2, 3])
                sc.__exit__(None, None, None)
                sc = tc.nc.named_scope("D1"); sc.__enter__()
                for s in range(8, 16):
                    scores_s(s)
                sc.__exit__(None, None, None)
                sc = tc.nc.named_scope("D2"); sc.__enter__()
                attv_chunk(1, [0, 1, 2, 3, 4, 5, 6, 7])
                sc.__exit__(None, None, None)

                # ============================================================
                # Phase E: normalize + output projection
                # ============================================================
                sc = tc.nc.named_scope("E"); sc.__enter__()
                nc.vector.tensor_copy(rs_sb[:], rs_ps[:])
                nc.vector.reciprocal(rs_sb[:], rs_sb[:])
                for ot in range(NT):
                    nc.vector.tensor_mul(Oacc[:, ot, :], Oacc[:, ot, :], rs_sb[:])
                    nc.scalar.activation(
                        Opair[:, 0, ot, :], Oacc[:, ot, :], AF.Identity
                    )
                    nc.vector.tensor_sub(
                        Opair[:, 1, ot, :], Oacc[:, ot, :], Opair[:, 0, ot, :]
                    )

                with tc.tile_pool(name="evac", bufs=3) as evac:
                    for o2 in range(NT):
                        osl = slice(o2 * P, (o2 + 1) * P)
                        for ci, (cs, ce) in enumerate(((0, 512), (512, 1024))):
                            ps = pp.tile([P, 512], FP, tag="ps")
                            first = True
                            for t in range(NT // 2):
                                nc.tensor.matmul(
                                    ps[:],
                                    lhsT=wp[:, 1, 2 * t : 2 * t + 2, osl],
                                    rhs=Opair[:, 0, 2 * t : 2 * t + 2, cs:ce],
                                    start=first,
                                    stop=False,
                                    perf_mode=DR,
                                )
                                first = False
                            for ct in range(NT):
                                nc.tensor.matmul(
                                    ps[:],
                                    lhsT=wp[:, 0:2, ct, osl],
                                    rhs=Opair[:, 0:2, ct, cs:ce],
                                    start=False,
                                    stop=(ct == NT - 1),
                                    perf_mode=DR,
                                )
                            ev = evac.tile([P, 512], BF, tag="evy")
                            nc.scalar.activation(
                                ev[:], ps[:], AF.Identity, bias=bias_sb[:, o2 : o2 + 1]
                            )
                            nc.sync.dma_start(
                                yT[(o2 * 2 + ci) * P : (o2 * 2 + ci + 1) * P, :],
                                ev[:],
                            )
                sc.__exit__(None, None, None)

    _split_waits(nc)
    return nc


_NC_CACHE = None


def _get_nc():
    global _NC_CACHE
    if _NC_CACHE is None:
        _NC_CACHE = _build_nc()
    return _NC_CACHE


def _pair(a, order="hl"):
    """Decompose fp32 array -> fp8 (hi, lo) or (lo, hi) pair along new axis 1.
    a: [P, ...]; returns [P, 2, ...] float8_e4m3."""
    import ml_dtypes

    a = np.asarray(a, dtype=np.float32)
    hi = a.astype(ml_dtypes.float8_e4m3)
    lo = (a - hi.astype(np.float32)).astype(ml_dtypes.float8_e4m3)
    pair = (hi, lo) if order == "hl" else (lo, hi)
    return np.ascontiguousarray(np.stack(pair, axis=1))


def _tile_major(m):
    """[C_in, N] -> [P, C_in//P, N] with partition dim first."""
    cin, n = m.shape
    return np.ascontiguousarray(m.reshape(cin // P, P, n).transpose(1, 0, 2))


def make_in_maps(x, w_qkv, b_qkv, w_proj, b_proj):
    """Host-side prep: shard + transpose + fp8-decompose inputs for 8 cores."""
    x = np.asarray(x, dtype=np.float32)
    w_qkv = np.asarray(w_qkv, dtype=np.float32)
    b_qkv = np.asarray(b_qkv, dtype=np.float32)
    w_proj = np.asarray(w_proj, dtype=np.float32)
    b_proj = np.asarray(b_proj, dtype=np.float32)
    import ml_dtypes

    s = 1.0 / np.sqrt(np.float32(C))

    # weights, stored (lo, hi), layout [P cpart, 2, NT csub, C out]
    wq = _pair(_tile_major((w_qkv[0:C] * s).T), "lh")
    wk = _pair(_tile_major(w_qkv[C : 2 * C].T), "lh")
    wv = _pair(_tile_major(w_qkv[2 * C : 3 * C].T), "lh")
    wp = _pair(_tile_major(w_proj.T), "lh")
    bqp = _pair(_tile_major((b_qkv[0:C] * s).reshape(C, 1)), "hl")

    bv = b_qkv[2 * C : 3 * C]
    beff = (b_proj + w_proj @ bv).reshape(NT, P).T
    bias = np.concatenate(
        [beff, np.full((P, 1), -np.log(ASCALE), np.float32)], axis=1
    ).astype(np.float32)

    ones = np.ones((P, 2, P), dtype=np.float32).astype(ml_dtypes.float8_e4m3)

    # S^T mask tiles: partition = kv j (within subtile), free = query i
    triu = np.triu(np.ones((P, P), dtype=np.float32))
    trilm = np.where(triu > 0, 0.0, NEG).astype(np.float32)
    zeros = np.zeros((P, P), dtype=np.float32)
    negs = np.full((P, P), NEG, dtype=np.float32)

    shared = dict(
        wq_in=wq, wk_in=wk, wv_in=wv, wp_in=wp, bqp_in=bqp,
        bias_in=bias, ones_in=ones,
    )
    in_maps = []
    for core in range(8):
        b, h = core // 2, core % 2
        xb = x[b]  # [T, C]
        qrows = np.concatenate(
            [xb[(2 * bg + h) * 256 : (2 * bg + h + 1) * 256] for bg in range(4)],
            axis=0,
        )
        m = np.stack(
            [
                trilm if h == 0 else zeros,   # m1d
                negs if h == 0 else zeros,    # m1f
                negs if h == 0 else trilm,    # m2d
            ],
            axis=1,
        )
        in_maps.append(
            dict(
                shared,
                xq_in=_pair(_tile_major(qrows.T), "hl"),
                xo_in=_pair(_tile_major(xb[0:H].T), "hl"),
                xx_in=_pair(_tile_major(xb[H : 2 * H].T), "hl"),
                masks_in=np.ascontiguousarray(m),
            )
        )
    return in_maps


def assemble_output(results):
    B = 4
    y = np.empty((B, 2 * H, C), dtype=np.float32)
    for core in range(8):
        b, h = core // 2, core % 2
        yt = results[core]["yT"].astype(np.float32).reshape(NT, 2, P, 512)
        blk = yt.transpose(1, 3, 0, 2).reshape(H, C)
        blk4 = blk.reshape(4, 256, C)
        for bg in range(4):
            g = 2 * bg + h
            y[b, g * 256 : (g + 1) * 256, :] = blk4[bg]
    return y


def kernel(x, w_qkv, b_qkv, w_proj, b_proj):
    from concourse.bass_utils import run_bass_kernel_spmd

    nc = _get_nc()
    in_maps = make_in_maps(x, w_qkv, b_qkv, w_proj, b_proj)
    res = run_bass_kernel_spmd(nc, in_maps, list(range(8)))
    return assemble_output(res.results)


# revision 7
# speedup vs baseline: 1.2719x; 1.1216x over previous
"""Single-head causal attention (B=4, T=2048, C=1024) on 8 trn2 NeuronCores.

Sharding: 8 shards = (batch b in 0..3) x (query interleave h in 0..1), same
balanced interleaved-256 query split as the fp32r baseline: core h of a pair
takes global 256-row query blocks {2*bg+h}, so the causal triangle is
balanced across the pair. One SPMD instruction stream; all per-core
variation is data (gathered x slices + three [128,128] mask tiles).

All matmuls run as compensated fp8e4m3 DoubleRow pairs. Each operand is
decomposed as v = hi + lo (hi = fp8(v), lo = fp8(v - hi)); a product
x*w = xh*wh + (xh*wl + xl*wh) keeps ~bf16 accuracy (residual ~0.1%) while
DoubleRow processes TWO 128-deep contraction planes per instruction at 0.5
cycles/row -- 4x the fp32r/bf16 rate, so the compensated triple costs 0.75x
of the bf16-equivalent. Host-side tensors (x, weights) are decomposed on
the host; device-computed tensors (q, k, v, exp-scores, attention out) are
decomposed with a cast + subtract pass (Act/DVE/Pool engines, all far under
the PE roofline).

Algebraic folds:
  - k bias dropped entirely: softmax over kv positions is invariant to
    per-query constants, and (q+bq).(k+bk) - (q+bq).k is constant per row.
  - q bias folded into the exp: s_ij = q~_i.k_j + (bq~.k_j), the second
    term is per-kv-position, computed on device as a tiny N=1 DoubleRow
    matmul chain (bqk), and applied as the Exp activation bias together
    with -ln(32).
  - exp(s)/32 stored instead of exp(s) so fp8's 240 max is never hit
    (scores ~N(0,1); the 1/32 cancels between att@V and the rowsum).
  - v bias folded into the output bias (beff = b_proj + w_proj @ b_v).
  - 1/sqrt(C) folded into wq/bq host-side.

fp8 halves SBUF, so unlike the baseline there is NO DRAM spill of kv half 1
(phases C/skT/sV are gone): kT/V for all 2048 kv positions, qT, AT and the
fp32 O accumulator are all resident.

Comp-plane storage convention (so compensated cross terms pair cleanly):
"moving-side" tensors (x, qT, AT, Opair, bq) store (hi, lo); "stationary
side" (wk, wq, wv, wp, kT, V) store (lo, hi). A cross op then reads
lhsT[:, 0:2] x rhs[:, 0:2] = wl*xh + wh*xl directly; hi*hi ops index plane
1 of the stationary and plane 0 of the moving tensor, pairing adjacent
contraction subtiles instead.
"""

import sys

sys.path.insert(0, "/opt/trn_rl_repo")

import numpy as np

import concourse.bass as bass
import concourse.tile as tile
from concourse import mybir
from concourse.vector_clock import ScopedClock

FP = mybir.dt.float32
BF = mybir.dt.bfloat16
F8 = mybir.dt.float8e4
AF = mybir.ActivationFunctionType
DR = mybir.MatmulPerfMode.DoubleRow

P = 128
C = 1024  # embed dim
H = 1024  # query rows per core
TKV = 2048  # kv length
NT = C // P  # 8 c-subtiles
NKV = TKV // P  # 16 kv-subtiles
NEG = -1.0e9
ASCALE = 32.0  # exp(s)/ASCALE stored in fp8

_MAX_WAITS = 1


class _TC(tile.TileContext):
    """TileContext whose tail drain puts its global-clock waits on a nop
    (walrus rejects multi-wait Drain); excess waits are split by
    _split_waits() afterwards."""

    def _drain_and_barrier(self, tick_clock, wait_clock):
        nop_inst = self.nc.sync.nop(nofuse=True, hint="pre_drain_waits")
        wait_clock.add_sem_waits(
            nop_inst.ins, ScopedClock({None: tick_clock.global_clock})
        )
        self.nc.sync.drain()
        self.nc.all_engine_barrier()
        assert self.sems is not None
        popped = self.nc._tile_sem_poison_stack.pop()
        assert popped is self._sem_poison
        self.nc.clear_and_free_semaphores(list(self.sems.allocated().values()))
        self.nc.all_engine_barrier()


def _split_waits(nc, max_waits=_MAX_WAITS):
    """The walrus shipped here rejects instructions carrying more than
    `max_waits` sync waits. Move excess waits onto injected nops placed
    immediately before the instruction on the same engine."""
    import copy

    template = nc.sync.nop(nofuse=True, hint="waitsplit_template").ins
    counter = [0]

    def make_nop(engine, waits):
        nop = copy.deepcopy(template)
        counter[0] += 1
        nop.name = f"I-wsplit-{counter[0]}"
        nop.engine = engine
        nop.sync_info = mybir.SyncInfo(on_wait=list(waits), on_update=[])
        return nop

    f = nc.m.functions[0]
    for bb in f.blocks:
        insts = bb.instructions
        if not any(
            i.sync_info and i.sync_info.on_wait and len(i.sync_info.on_wait) > max_waits
            for i in insts
        ):
            continue
        newlist = []
        for inst in insts:
            si = inst.sync_info
            if si and si.on_wait and len(si.on_wait) > max_waits:
                if inst.name == template.name:
                    newlist.append(inst)
                    continue
                waits = list(si.on_wait)
                del si.on_wait[max_waits:]
                rest = waits[max_waits:]
                while rest:
                    newlist.append(make_nop(inst.engine, rest[:max_waits]))
                    rest = rest[max_waits:]
            newlist.append(inst)
        bb.instructions[:] = newlist


# Causal structure for the interleaved-256 query sharding, over 16 kv
# 128-subtiles. Query slots bg=0..3 hold global 256-row blocks g=2*bg+h.
# For kv subtile s, valid query cols start at LO16[s]*128; mask tiles
# (data-encoded per core) are added at the listed 128-col block positions.
LO16 = [0, 0, 0, 1, 2, 2, 2, 3, 4, 4, 4, 5, 6, 6, 6, 7]
_MASKS8 = [
    [(0, 0)],            # (128-block, mask index) ; 0=m1d 1=m1f 2=m2d
    [(0, 1), (1, 0)],
    [(0, 2), (1, 1)],
    [(1, 2)],
    [(2, 0)],
    [(2, 1), (3, 0)],
    [(2, 2), (3, 1)],
    [(3, 2)],
]
MASKS16 = [
    [((s // 8) * 4 + off, mi) for off, mi in _MASKS8[s % 8]] for s in range(16)
]
# pair-aligned lo (attv pairs kv subtiles (2p, 2p+1))
LOP16 = [LO16[s] - (LO16[s] % 2) for s in range(16)]


def _chunks512(lo, hi):
    """Split [lo, hi) at absolute multiples of 512."""
    out = []
    while lo < hi:
        ce = min((lo // 512 + 1) * 512, hi)
        out.append((lo, ce))
        lo = ce
    return out


def _build_nc():
    nc = bass.Bass("TRN2", target_bir_lowering=False, debug=False)

    xq_in = nc.dram_tensor("xq_in", [P, 2, NT, H], F8, kind="ExternalInput").ap()
    xo_in = nc.dram_tensor("xo_in", [P, 2, NT, H], F8, kind="ExternalInput").ap()
    xx_in = nc.dram_tensor("xx_in", [P, 2, NT, H], F8, kind="ExternalInput").ap()
    wk_in = nc.dram_tensor("wk_in", [P, 2, NT, C], F8, kind="ExternalInput").ap()
    wq_in = nc.dram_tensor("wq_in", [P, 2, NT, C], F8, kind="ExternalInput").ap()
    wv_in = nc.dram_tensor("wv_in", [P, 2, NT, C], F8, kind="ExternalInput").ap()
    wp_in = nc.dram_tensor("wp_in", [P, 2, NT, C], F8, kind="ExternalInput").ap()
    bqp_in = nc.dram_tensor("bqp_in", [P, 2, NT, 1], F8, kind="ExternalInput").ap()
    ones_in = nc.dram_tensor("ones_in", [P, 2, P], F8, kind="ExternalInput").ap()
    masks_in = nc.dram_tensor("masks_in", [P, 3, P], FP, kind="ExternalInput").ap()
    # beff (8 cols) | -ln(ASCALE) (1 col)
    bias_in = nc.dram_tensor("bias_in", [P, NT + 1], FP, kind="ExternalInput").ap()
    # output, (o2-tile, chunk)-major, bf16; host reassembles + upcasts
    yT = nc.dram_tensor("yT", [NT * 2 * P, 512], BF, kind="ExternalOutput").ap()

    with _TC(nc) as tc:
        with (
            tc.tile_pool(name="misc", bufs=1) as misc,
            tc.tile_pool(name="kqv", bufs=1) as kqv,
            tc.tile_pool(name="psum", bufs=5, space="PSUM") as pp,
            tc.tile_pool(name="psum_rs", bufs=1, space="PSUM") as pp_rs,
        ):
            ones_sb = misc.tile([P, 2, P], F8, tag="ones")
            masks = misc.tile([P, 3, P], FP, tag="masks")
            bias_sb = misc.tile([P, NT + 1], FP, tag="bias")
            bqp = misc.tile([P, 2, NT, 1], F8, tag="bqp")
            bqk_sb = misc.tile([P, NKV], FP, tag="bqk")

            # persistent fp8 pair tensors (comp order noted)
            kT = kqv.tile([P, 2, NT, TKV], F8, tag="kT")   # (lo, hi)
            qT = kqv.tile([P, 2, NT, H], F8, tag="qT")     # (hi, lo)
            V = kqv.tile([P, 2, NKV, C], F8, tag="V")      # (lo, hi)
            wp = kqv.tile([P, 2, NT, C], F8, tag="wp")     # (lo, hi)

            # =============================================================
            # Phase A: projections
            # =============================================================
            with tc.tile_pool(name="xw", bufs=1) as xw:
                xo = xw.tile([P, 2, NT, H], F8, tag="xo")
                xx = xw.tile([P, 2, NT, H], F8, tag="xx")
                xq = xw.tile([P, 2, NT, H], F8, tag="xq")
                wk = xw.tile([P, 2, NT, C], F8, tag="wk")
                wq = xw.tile([P, 2, NT, C], F8, tag="wq")
                wv = xw.tile([P, 2, NT, C], F8, tag="wv")

                # fine-grained first loads so kproj starts ASAP; ones
                # first to feed the PE p-state warmup
                nc.sync.dma_start(ones_sb[:], ones_in[:])
                nc.sync.dma_start(xo[:, :, :, 0:512], xo_in[:, :, :, 0:512])
                for ot in range(NT):
                    nc.sync.dma_start(
                        wk[:, :, :, ot * P : (ot + 1) * P],
                        wk_in[:, :, :, ot * P : (ot + 1) * P],
                    )
                nc.sync.dma_start(xo[:, :, :, 512:1024], xo_in[:, :, :, 512:1024])
                nc.sync.dma_start(xx[:], xx_in[:])
                nc.sync.dma_start(xq[:], xq_in[:])
                nc.sync.dma_start(wq[:], wq_in[:])
                nc.sync.dma_start(wv[:], wv_in[:])
                nc.sync.dma_start(masks[:], masks_in[:])
                nc.sync.dma_start(bias_sb[:], bias_in[:])
                nc.sync.dma_start(bqp[:], bqp_in[:])
                nc.sync.dma_start(wp[:], wp_in[:])

                # ~3us of junk DoubleRow matmuls on the ones tile: ramps the
                # PE p-state while the first x/w DMAs are still in flight
                wps = pp.tile([P, 512], FP, tag="ps", name="wps")
                for _ in range(60):
                    nc.tensor.matmul(
                        wps[:, 0:P],
                        lhsT=ones_sb[:],
                        rhs=ones_sb[:, :, :],
                        start=True,
                        stop=True,
                        perf_mode=DR,
                        skip_group_check=True,
                    )

                xhalf = [xo, xx]

                def mm12(ps, w, x, osl, cs, ce, n_start=True, n_stop=True):
                    """12-op compensated group: out[osl, cs:ce] += w.T @ x.
                    w stored (lo,hi), x stored (hi,lo); contraction over all
                    NT c-subtiles."""
                    first = [n_start]
                    for t in range(NT // 2):
                        nc.tensor.matmul(
                            ps[:, : ce - cs],
                            lhsT=w[:, 1, 2 * t : 2 * t + 2, osl],
                            rhs=x[:, 0, 2 * t : 2 * t + 2, cs:ce],
                            start=first[0],
                            stop=False,
                            perf_mode=DR,
                        )
                        first[0] = False
                    for ct in range(NT):
                        nc.tensor.matmul(
                            ps[:, : ce - cs],
                            lhsT=w[:, 0:2, ct, osl],
                            rhs=x[:, 0:2, ct, cs:ce],
                            start=False,
                            stop=(n_stop and ct == NT - 1),
                            perf_mode=DR,
                        )

                # ---- k projection (no bias; softmax-invariant) ----------
                sc = tc.nc.named_scope("A_k"); sc.__enter__()
                for half, cs in ((0, 0), (0, 512), (1, 0), (1, 512)):
                    xh = xhalf[half]
                    ce = cs + 512
                    for ot in range(NT):
                        osl = slice(ot * P, (ot + 1) * P)
                        ps = pp.tile([P, 512], FP, tag="ps")
                        mm12(ps, wk, xh, osl, cs, ce)
                        g0 = half * H + cs
                        nc.scalar.activation(
                            kT[:, 1, ot, g0 : g0 + 512], ps[:], AF.Identity
                        )
                        nc.vector.tensor_sub(
                            kT[:, 0, ot, g0 : g0 + 512],
                            ps[:],
                            kT[:, 1, ot, g0 : g0 + 512],
                        )
                sc.__exit__(None, None, None)

                # ---- bqk: per-kv-position q-bias term (bq~ . k_j) -------
                sc = tc.nc.named_scope("A_bqk"); sc.__enter__()
                psb_pool = tc.tile_pool(name="psb", bufs=1, space="PSUM")
                ppb = psb_pool.__enter__()
                ps_b = ppb.tile([P, NKV], FP, tag="psb")
                nop = 0
                for kvt in range(NKV):
                    ksl = slice(kvt * P, (kvt + 1) * P)
                    for t in range(NT // 2):
                        nc.tensor.matmul(
                            ps_b[:, kvt : kvt + 1],
                            lhsT=kT[:, 1, 2 * t : 2 * t + 2, ksl],
                            rhs=bqp[:, 0, 2 * t : 2 * t + 2, :],
                            start=(nop == 0),
                            stop=False,
                            perf_mode=DR,
                            skip_group_check=True,
                        )
                        nop += 1
                    for ct in range(NT):
                        nop += 1
                        nc.tensor.matmul(
                            ps_b[:, kvt : kvt + 1],
                            lhsT=kT[:, 0:2, ct, ksl],
                            rhs=bqp[:, 0:2, ct, :],
                            start=False,
                            stop=(nop == 12 * NKV),
                            perf_mode=DR,
                            skip_group_check=True,
                        )
                # bqk_sb = bqk - ln(ASCALE): the Exp bias for each kv row
                nc.scalar.activation(
                    bqk_sb[:], ps_b[:], AF.Identity, bias=bias_sb[:, NT : NT + 1]
                )
                psb_pool.__exit__(None, None, None)
                sc.__exit__(None, None, None)

                # ---- v projection (x stationary, w moving; no bias) -----
                sc = tc.nc.named_scope("A_v"); sc.__enter__()
                for half in range(2):
                    xh = xhalf[half]
                    for tt in range(NT):
                        ts_g = half * NT + tt
                        tsl = slice(tt * P, (tt + 1) * P)
                        for cs, ce in ((0, 512), (512, 1024)):
                            ps = pp.tile([P, 512], FP, tag="ps")
                            first = True
                            for t in range(NT // 2):
                                nc.tensor.matmul(
                                    ps[:],
                                    lhsT=xh[:, 0, 2 * t : 2 * t + 2, tsl],
                                    rhs=wv[:, 1, 2 * t : 2 * t + 2, cs:ce],
                                    start=first,
                                    stop=False,
                                    perf_mode=DR,
                                )
                                first = False
                            for ct in range(NT):
                                nc.tensor.matmul(
                                    ps[:],
                                    lhsT=xh[:, 0:2, ct, tsl],
                                    rhs=wv[:, 0:2, ct, cs:ce],
                                    start=False,
                                    stop=(ct == NT - 1),
                                    perf_mode=DR,
                                )
                            nc.scalar.activation(
                                V[:, 1, ts_g, cs:ce], ps[:], AF.Identity
                            )
                            nc.vector.tensor_sub(
                                V[:, 0, ts_g, cs:ce], ps[:], V[:, 1, ts_g, cs:ce]
                            )
                sc.__exit__(None, None, None)

                # ---- q projection (scaled wq; bias via bqk) -------------
                sc = tc.nc.named_scope("A_q"); sc.__enter__()
                for ot in range(NT):
                    osl = slice(ot * P, (ot + 1) * P)
                    for cs, ce in ((0, 512), (512, 1024)):
                        ps = pp.tile([P, 512], FP, tag="ps")
                        mm12(ps, wq, xq, osl, cs, ce)
                        nc.scalar.activation(
                            qT[:, 0, ot, cs:ce], ps[:], AF.Identity
                        )
                        nc.vector.tensor_sub(
                            qT[:, 1, ot, cs:ce], ps[:], qT[:, 0, ot, cs:ce]
                        )
                sc.__exit__(None, None, None)

            # =============================================================
            # Phases B-D (attention): xw freed; AT/Oacc/Opair reuse space
            # =============================================================
            with (
                tc.tile_pool(name="attn", bufs=1) as ab,
                tc.tile_pool(name="efp", bufs=3) as efp,
            ):
                AT = ab.tile([P, 2, NKV, H], F8, tag="AT")   # (hi, lo)
                Oacc = ab.tile([P, NT, H], FP, tag="Oacc")
                Opair = ab.tile([P, 2, NT, H], F8, tag="Op")  # (hi, lo)
                rs_sb = ab.tile([P, H], FP, tag="rs_sb")
                rs_ps = pp_rs.tile([P, H], FP, tag="rs")

                ef_cur = [None]

                def scores_s(s):
                    lo_s = LO16[s] * P
                    lo_p = LOP16[s] * P
                    if s % 2 == 0:
                        ef_cur[0] = efp.tile([P, 2, H], BF, tag="ef", name=f"ef{s}")
                    ef = ef_cur[0]
                    chs = _chunks512(lo_p, H)
                    pss = [
                        pp.tile([P, ce - cs], FP, tag="ps", name=f"pss{s}_{cs}")
                        for cs, ce in chs
                    ]
                    # ct-outer so each stationary kT slice loads once
                    nop, last = 0, 12 * len(chs)
                    for t in range(NT // 2):
                        for ps, (cs, ce) in zip(pss, chs):
                            mlo = max(cs, lo_s)
                            nc.tensor.matmul(
                                ps[:, mlo - cs : ce - cs],
                                lhsT=kT[:, 1, 2 * t : 2 * t + 2, s * P : (s + 1) * P],
                                rhs=qT[:, 0, 2 * t : 2 * t + 2, mlo:ce],
                                start=(nop < len(chs)),
                                stop=False,
                                perf_mode=DR,
                                skip_group_check=True,
                            )
                            nop += 1
                    for ct in range(NT):
                        for ps, (cs, ce) in zip(pss, chs):
                            mlo = max(cs, lo_s)
                            nop += 1
                            nc.tensor.matmul(
                                ps[:, mlo - cs : ce - cs],
                                lhsT=kT[:, 0:2, ct, s * P : (s + 1) * P],
                                rhs=qT[:, 0:2, ct, mlo:ce],
                                start=False,
                                stop=(nop > last - len(chs)),
                                perf_mode=DR,
                                skip_group_check=True,
                            )
                    # dead sliver [lo_p, lo_s): exp(-1e9) = 0 keeps the fp8
                    # pair exactly zero there so paired attv ops read zeros
                    if lo_s > lo_p:
                        nc.vector.memset(pss[0][:, 0 : lo_s - lo_p], NEG)
                    for ps, (cs, ce) in zip(pss, chs):
                        for blk, mi in MASKS16[s]:
                            a = blk * P
                            if cs <= a < ce:
                                nc.vector.tensor_add(
                                    ps[:, a - cs : a - cs + P],
                                    ps[:, a - cs : a - cs + P],
                                    masks[:, mi, :],
                                )
                        nc.scalar.activation(
                            ef[:, s % 2, cs:ce],
                            ps[:],
                            AF.Exp,
                            bias=bqk_sb[:, s : s + 1],
                        )
                    if s % 2 == 1:
                        # pair complete: decompose exp into the AT fp8 pair
                        nc.gpsimd.tensor_copy(
                            AT[:, 0, s - 1 : s + 1, lo_p:H], ef[:, :, lo_p:H]
                        )
                        nc.gpsimd.tensor_sub(
                            AT[:, 1, s - 1 : s + 1, lo_p:H],
                            ef[:, :, lo_p:H],
                            AT[:, 0, s - 1 : s + 1, lo_p:H],
                        )
                        # rowsum for both subtiles of the pair
                        for sj in (s - 1, s):
                            lo_j = LOP16[sj] * P
                            for cs, ce in _chunks512(lo_j, H):
                                nc.tensor.matmul(
                                    rs_ps[:, cs:ce],
                                    lhsT=ones_sb[:],
                                    rhs=AT[:, 0:2, sj, cs:ce],
                                    start=(sj == 0),
                                    stop=(
                                        (cs < 512 and sj == 7)
                                        or (cs >= 512 and sj == NKV - 1)
                                    ),
                                    perf_mode=DR,
                                    skip_group_check=True,
                                )

                def attv_chunk(ci, pairs):
                    cs, ce = ci * 512, (ci + 1) * 512
                    for ot in range(NT):
                        osl = slice(ot * P, (ot + 1) * P)
                        ps = pp.tile([P, 512], FP, tag="ps")
                        nops = 3 * len(pairs)
                        nop = 0
                        for p in pairs:
                            plo = max(cs, LOP16[2 * p] * P)
                            nc.tensor.matmul(
                                ps[:, plo - cs : 512],
                                lhsT=V[:, 1, 2 * p : 2 * p + 2, osl],
                                rhs=AT[:, 0, 2 * p : 2 * p + 2, plo:ce],
                                start=(nop == 0),
                                stop=False,
                                perf_mode=DR,
                                skip_group_check=True,
                            )
                            nop += 1
                            for sj in (2 * p, 2 * p + 1):
                                nop += 1
                                nc.tensor.matmul(
                                    ps[:, plo - cs : 512],
                                    lhsT=V[:, 0:2, sj, osl],
                                    rhs=AT[:, 0:2, sj, plo:ce],
                                    start=False,
                                    stop=(nop == nops),
                                    perf_mode=DR,
                                    skip_group_check=True,
                                )
                        nc.vector.tensor_copy(Oacc[:, ot, cs:ce], ps[:])

                def normalize_chunk(ci):
                    cs, ce = ci * 512, (ci + 1) * 512
                    nc.vector.tensor_copy(rs_sb[:, cs:ce], rs_ps[:, cs:ce])
                    nc.vector.reciprocal(rs_sb[:, cs:ce], rs_sb[:, cs:ce])
                    for ot in range(NT):
                        nc.vector.tensor_mul(
                            Oacc[:, ot, cs:ce], Oacc[:, ot, cs:ce], rs_sb[:, cs:ce]
                        )
                        nc.scalar.activation(
                            Opair[:, 0, ot, cs:ce], Oacc[:, ot, cs:ce], AF.Identity
                        )
                        nc.vector.tensor_sub(
                            Opair[:, 1, ot, cs:ce],
                            Oacc[:, ot, cs:ce],
                            Opair[:, 0, ot, cs:ce],
                        )

                def oproj_chunk(ci, evac):
                    cs, ce = ci * 512, (ci + 1) * 512
                    for o2 in range(NT):
                        osl = slice(o2 * P, (o2 + 1) * P)
                        ps = pp.tile([P, 512], FP, tag="ps")
                        first = True
                        for t in range(NT // 2):
                            nc.tensor.matmul(
                                ps[:],
                                lhsT=wp[:, 1, 2 * t : 2 * t + 2, osl],
                                rhs=Opair[:, 0, 2 * t : 2 * t + 2, cs:ce],
                                start=first,
                                stop=False,
                                perf_mode=DR,
                            )
                            first = False
                        for ct in range(NT):
                            nc.tensor.matmul(
                                ps[:],
                                lhsT=wp[:, 0:2, ct, osl],
                                rhs=Opair[:, 0:2, ct, cs:ce],
                                start=False,
                                stop=(ct == NT - 1),
                                perf_mode=DR,
                            )
                        ev = evac.tile([P, 512], BF, tag="evy", name=f"evy{ci}_{o2}")
                        nc.scalar.activation(
                            ev[:], ps[:], AF.Identity, bias=bias_sb[:, o2 : o2 + 1]
                        )
                        nc.sync.dma_start(
                            yT[(o2 * 2 + ci) * P : (o2 * 2 + ci + 1) * P, :],
                            ev[:],
                        )

                sc = tc.nc.named_scope("B1"); sc.__enter__()
                for s in range(16):
                    scores_s(s)
                sc.__exit__(None, None, None)
                with tc.tile_pool(name="evac", bufs=3) as evac:
                    sc = tc.nc.named_scope("B2"); sc.__enter__()
                    attv_chunk(0, [0, 1, 2, 3])
                    normalize_chunk(0)
                    sc.__exit__(None, None, None)
                    sc = tc.nc.named_scope("D2"); sc.__enter__()
                    attv_chunk(1, [0, 1, 2, 3, 4, 5, 6, 7])
                    normalize_chunk(1)
                    sc.__exit__(None, None, None)
                    sc = tc.nc.named_scope("E"); sc.__enter__()
                    oproj_chunk(0, evac)
                    oproj_chunk(1, evac)
                    sc.__exit__(None, None, None)

    _split_waits(nc)
    return nc


_NC_CACHE = None


def _get_nc():
    global _NC_CACHE
    if _NC_CACHE is None:
        _NC_CACHE = _build_nc()
    return _NC_CACHE


def _pair(a, order="hl"):
    """Decompose fp32 array -> fp8 (hi, lo) or (lo, hi) pair along new axis 1.
    a: [P, ...]; returns [P, 2, ...] float8_e4m3."""
    import ml_dtypes

    a = np.asarray(a, dtype=np.float32)
    hi = a.astype(ml_dtypes.float8_e4m3)
    lo = (a - hi.astype(np.float32)).astype(ml_dtypes.float8_e4m3)
    pair = (hi, lo) if order == "hl" else (lo, hi)
    return np.ascontiguousarray(np.stack(pair, axis=1))


def _tile_major(m):
    """[C_in, N] -> [P, C_in//P, N] with partition dim first."""
    cin, n = m.shape
    return np.ascontiguousarray(m.reshape(cin // P, P, n).transpose(1, 0, 2))


def make_in_maps(x, w_qkv, b_qkv, w_proj, b_proj):
    """Host-side prep: shard + transpose + fp8-decompose inputs for 8 cores."""
    x = np.asarray(x, dtype=np.float32)
    w_qkv = np.asarray(w_qkv, dtype=np.float32)
    b_qkv = np.asarray(b_qkv, dtype=np.float32)
    w_proj = np.asarray(w_proj, dtype=np.float32)
    b_proj = np.asarray(b_proj, dtype=np.float32)
    import ml_dtypes

    s = 1.0 / np.sqrt(np.float32(C))

    # weights, stored (lo, hi), layout [P cpart, 2, NT csub, C out]
    wq = _pair(_tile_major((w_qkv[0:C] * s).T), "lh")
    wk = _pair(_tile_major(w_qkv[C : 2 * C].T), "lh")
    wv = _pair(_tile_major(w_qkv[2 * C : 3 * C].T), "lh")
    wp = _pair(_tile_major(w_proj.T), "lh")
    bqp = _pair(_tile_major((b_qkv[0:C] * s).reshape(C, 1)), "hl")

    bv = b_qkv[2 * C : 3 * C]
    beff = (b_proj + w_proj @ bv).reshape(NT, P).T
    bias = np.concatenate(
        [beff, np.full((P, 1), -np.log(ASCALE), np.float32)], axis=1
    ).astype(np.float32)

    ones = np.ones((P, 2, P), dtype=np.float32).astype(ml_dtypes.float8_e4m3)

    # S^T mask tiles: partition = kv j (within subtile), free = query i
    triu = np.triu(np.ones((P, P), dtype=np.float32))
    trilm = np.where(triu > 0, 0.0, NEG).astype(np.float32)
    zeros = np.zeros((P, P), dtype=np.float32)
    negs = np.full((P, P), NEG, dtype=np.float32)

    shared = dict(
        wq_in=wq, wk_in=wk, wv_in=wv, wp_in=wp, bqp_in=bqp,
        bias_in=bias, ones_in=ones,
    )
    in_maps = []
    for core in range(8):
        b, h = core // 2, core % 2
        xb = x[b]  # [T, C]
        qrows = np.concatenate(
            [xb[(2 * bg + h) * 256 : (2 * bg + h + 1) * 256] for bg in range(4)],
            axis=0,
        )
        m = np.stack(
            [
                trilm if h == 0 else zeros,   # m1d
                negs if h == 0 else zeros,    # m1f
                negs if h == 0 else trilm,    # m2d
            ],
            axis=1,
        )
        in_maps.append(
            dict(
                shared,
                xq_in=_pair(_tile_major(qrows.T), "hl"),
                xo_in=_pair(_tile_major(xb[0:H].T), "hl"),
                xx_in=_pair(_tile_major(xb[H : 2 * H].T), "hl"),
                masks_in=np.ascontiguousarray(m),
            )
        )
    return in_maps


def assemble_output(results):
    B = 4
    y = np.empty((B, 2 * H, C), dtype=np.float32)
    for core in range(8):
        b, h = core // 2, core % 2
        yt = results[core]["yT"].astype(np.float32).reshape(NT, 2, P, 512)
        blk = yt.transpose(1, 3, 0, 2).reshape(H, C)
        blk4 = blk.reshape(4, 256, C)
        for bg in range(4):
            g = 2 * bg + h
            y[b, g * 256 : (g + 1) * 256, :] = blk4[bg]
    return y


def kernel(x, w_qkv, b_qkv, w_proj, b_proj):
    from concourse.bass_utils import run_bass_kernel_spmd

    nc = _get_nc()
    in_maps = make_in_maps(x, w_qkv, b_qkv, w_proj, b_proj)
    res = run_bass_kernel_spmd(nc, in_maps, list(range(8)))
    return assemble_output(res.results)


# revision 8
# speedup vs baseline: 1.3414x; 1.0547x over previous
"""Single-head causal attention (B=4, T=2048, C=1024) on 8 trn2 NeuronCores.

Sharding: 8 shards = (batch b in 0..3) x (query interleave h in 0..1), same
balanced interleaved-256 query split as the fp32r baseline: core h of a pair
takes global 256-row query blocks {2*bg+h}, so the causal triangle is
balanced across the pair. One SPMD instruction stream; all per-core
variation is data (gathered x slices + three [128,128] mask tiles).

All matmuls run as compensated fp8e4m3 DoubleRow pairs. Each operand is
decomposed as v = hi + lo (hi = fp8(v), lo = fp8(v - hi)); a product
x*w = xh*wh + (xh*wl + xl*wh) keeps ~bf16 accuracy (residual ~0.1%) while
DoubleRow processes TWO 128-deep contraction planes per instruction at 0.5
cycles/row -- 4x the fp32r/bf16 rate, so the compensated triple costs 0.75x
of the bf16-equivalent. Host-side tensors (x, weights) are decomposed on
the host; device-computed tensors (q, k, v, exp-scores, attention out) are
decomposed with a cast + subtract pass (Act/DVE/Pool engines, all far under
the PE roofline).

Algebraic folds:
  - k bias dropped entirely: softmax over kv positions is invariant to
    per-query constants, and (q+bq).(k+bk) - (q+bq).k is constant per row.
  - q bias folded into the exp: s_ij = q~_i.k_j + (bq~.k_j), the second
    term is per-kv-position, computed on device as a tiny N=1 DoubleRow
    matmul chain (bqk), and applied as the Exp activation bias together
    with -ln(32).
  - exp(s)/32 stored instead of exp(s) so fp8's 240 max is never hit
    (scores ~N(0,1); the 1/32 cancels between att@V and the rowsum).
  - v bias folded into the output bias (beff = b_proj + w_proj @ b_v).
  - 1/sqrt(C) folded into wq/bq host-side.

fp8 halves SBUF, so unlike the baseline there is NO DRAM spill of kv half 1
(phases C/skT/sV are gone): kT/V for all 2048 kv positions, qT, AT and the
fp32 O accumulator are all resident.

Comp-plane storage convention (so compensated cross terms pair cleanly):
"moving-side" tensors (x, qT, AT, Opair, bq) store (hi, lo); "stationary
side" (wk, wq, wv, wp, kT, V) store (lo, hi). A cross op then reads
lhsT[:, 0:2] x rhs[:, 0:2] = wl*xh + wh*xl directly; hi*hi ops index plane
1 of the stationary and plane 0 of the moving tensor, pairing adjacent
contraction subtiles instead.
"""

import sys

sys.path.insert(0, "/opt/trn_rl_repo")

import numpy as np

import concourse.bass as bass
import concourse.tile as tile
from concourse import mybir
from concourse.vector_clock import ScopedClock

FP = mybir.dt.float32
BF = mybir.dt.bfloat16
F8 = mybir.dt.float8e4
AF = mybir.ActivationFunctionType
DR = mybir.MatmulPerfMode.DoubleRow

P = 128
C = 1024  # embed dim
H = 1024  # query rows per core
TKV = 2048  # kv length
NT = C // P  # 8 c-subtiles
NKV = TKV // P  # 16 kv-subtiles
NEG = -1.0e9
ASCALE = 32.0  # exp(s)/ASCALE stored in fp8

_MAX_WAITS = 1


class _TC(tile.TileContext):
    """TileContext whose tail drain puts its global-clock waits on a nop
    (walrus rejects multi-wait Drain); excess waits are split by
    _split_waits() afterwards."""

    def _drain_and_barrier(self, tick_clock, wait_clock):
        nop_inst = self.nc.sync.nop(nofuse=True, hint="pre_drain_waits")
        wait_clock.add_sem_waits(
            nop_inst.ins, ScopedClock({None: tick_clock.global_clock})
        )
        self.nc.sync.drain()
        self.nc.all_engine_barrier()
        assert self.sems is not None
        popped = self.nc._tile_sem_poison_stack.pop()
        assert popped is self._sem_poison
        self.nc.clear_and_free_semaphores(list(self.sems.allocated().values()))
        self.nc.all_engine_barrier()


def _split_waits(nc, max_waits=_MAX_WAITS):
    """The walrus shipped here rejects instructions carrying more than
    `max_waits` sync waits. Move excess waits onto injected nops placed
    immediately before the instruction on the same engine."""
    import copy

    template = nc.sync.nop(nofuse=True, hint="waitsplit_template").ins
    counter = [0]

    def make_nop(engine, waits):
        nop = copy.deepcopy(template)
        counter[0] += 1
        nop.name = f"I-wsplit-{counter[0]}"
        nop.engine = engine
        nop.sync_info = mybir.SyncInfo(on_wait=list(waits), on_update=[])
        return nop

    f = nc.m.functions[0]
    for bb in f.blocks:
        insts = bb.instructions
        if not any(
            i.sync_info and i.sync_info.on_wait and len(i.sync_info.on_wait) > max_waits
            for i in insts
        ):
            continue
        newlist = []
        for inst in insts:
            si = inst.sync_info
            if si and si.on_wait and len(si.on_wait) > max_waits:
                if inst.name == template.name:
                    newlist.append(inst)
                    continue
                waits = list(si.on_wait)
                del si.on_wait[max_waits:]
                rest = waits[max_waits:]
                while rest:
                    newlist.append(make_nop(inst.engine, rest[:max_waits]))
                    rest = rest[max_waits:]
            newlist.append(inst)
        bb.instructions[:] = newlist


# Causal structure for the interleaved-256 query sharding, over 16 kv
# 128-subtiles. Query slots bg=0..3 hold global 256-row blocks g=2*bg+h.
# For kv subtile s, valid query cols start at LO16[s]*128; mask tiles
# (data-encoded per core) are added at the listed 128-col block positions.
LO16 = [0, 0, 0, 1, 2, 2, 2, 3, 4, 4, 4, 5, 6, 6, 6, 7]
_MASKS8 = [
    [(0, 0)],            # (128-block, mask index) ; 0=m1d 1=m1f 2=m2d
    [(0, 1), (1, 0)],
    [(0, 2), (1, 1)],
    [(1, 2)],
    [(2, 0)],
    [(2, 1), (3, 0)],
    [(2, 2), (3, 1)],
    [(3, 2)],
]
MASKS16 = [
    [((s // 8) * 4 + off, mi) for off, mi in _MASKS8[s % 8]] for s in range(16)
]
# pair-aligned lo (attv pairs kv subtiles (2p, 2p+1))
LOP16 = [LO16[s] - (LO16[s] % 2) for s in range(16)]


def _chunks512(lo, hi):
    """Split [lo, hi) at absolute multiples of 512."""
    out = []
    while lo < hi:
        ce = min((lo // 512 + 1) * 512, hi)
        out.append((lo, ce))
        lo = ce
    return out


def _build_nc():
    nc = bass.Bass("TRN2", target_bir_lowering=False, debug=False)

    xq_in = nc.dram_tensor("xq_in", [P, 2, NT, H], F8, kind="ExternalInput").ap()
    xo_in = nc.dram_tensor("xo_in", [P, 2, NT, H], F8, kind="ExternalInput").ap()
    xx_in = nc.dram_tensor("xx_in", [P, 2, NT, H], F8, kind="ExternalInput").ap()
    wk_in = nc.dram_tensor("wk_in", [P, 2, NT, C], F8, kind="ExternalInput").ap()
    wq_in = nc.dram_tensor("wq_in", [P, 2, NT, C], F8, kind="ExternalInput").ap()
    wv_in = nc.dram_tensor("wv_in", [P, 2, NT, C], F8, kind="ExternalInput").ap()
    wp_in = nc.dram_tensor("wp_in", [P, 2, NT, C], F8, kind="ExternalInput").ap()
    bqp_in = nc.dram_tensor("bqp_in", [P, 2, NT, 1], F8, kind="ExternalInput").ap()
    ones_in = nc.dram_tensor("ones_in", [P, 2, P], F8, kind="ExternalInput").ap()
    masks_in = nc.dram_tensor("masks_in", [P, 3, P], FP, kind="ExternalInput").ap()
    # beff (8 cols) | -ln(ASCALE) (1 col)
    bias_in = nc.dram_tensor("bias_in", [P, NT + 1], FP, kind="ExternalInput").ap()
    # output, (o2-tile, chunk)-major, bf16; host reassembles + upcasts
    yT = nc.dram_tensor("yT", [NT * 2 * P, 512], BF, kind="ExternalOutput").ap()

    with _TC(nc) as tc:
        with (
            tc.tile_pool(name="misc", bufs=1) as misc,
            tc.tile_pool(name="kqv", bufs=1) as kqv,
            tc.tile_pool(name="psum", bufs=5, space="PSUM") as pp,
            tc.tile_pool(name="psum_rs", bufs=1, space="PSUM") as pp_rs,
        ):
            ones_sb = misc.tile([P, 2, P], F8, tag="ones")
            masks = misc.tile([P, 3, P], FP, tag="masks")
            bias_sb = misc.tile([P, NT + 1], FP, tag="bias")
            bqp = misc.tile([P, 2, NT, 1], F8, tag="bqp")
            bqk_sb = misc.tile([P, NKV], FP, tag="bqk")

            # persistent fp8 pair tensors (comp order noted)
            kT = kqv.tile([P, 2, NT, TKV], F8, tag="kT")   # (lo, hi)
            qT = kqv.tile([P, 2, NT, H], F8, tag="qT")     # (hi, lo)
            V = kqv.tile([P, 2, NKV, C], F8, tag="V")      # (lo, hi)
            wp = kqv.tile([P, 2, NT, C], F8, tag="wp")     # (lo, hi)

            # =============================================================
            # Phase A: projections
            # =============================================================
            with tc.tile_pool(name="xw", bufs=1) as xw:
                xo = xw.tile([P, 2, NT, H], F8, tag="xo")
                xx = xw.tile([P, 2, NT, H], F8, tag="xx")
                xq = xw.tile([P, 2, NT, H], F8, tag="xq")
                wk = xw.tile([P, 2, NT, C], F8, tag="wk")
                wq = xw.tile([P, 2, NT, C], F8, tag="wq")
                wv = xw.tile([P, 2, NT, C], F8, tag="wv")

                # fine-grained first loads so kproj starts ASAP; ones
                # first to feed the PE p-state warmup
                nc.sync.dma_start(ones_sb[:], ones_in[:])
                nc.sync.dma_start(xo[:, :, :, 0:512], xo_in[:, :, :, 0:512])
                for ot in range(NT):
                    nc.sync.dma_start(
                        wk[:, :, :, ot * P : (ot + 1) * P],
                        wk_in[:, :, :, ot * P : (ot + 1) * P],
                    )
                nc.sync.dma_start(xo[:, :, :, 512:1024], xo_in[:, :, :, 512:1024])
                nc.sync.dma_start(xx[:], xx_in[:])
                nc.sync.dma_start(xq[:], xq_in[:])
                nc.sync.dma_start(wq[:], wq_in[:])
                nc.sync.dma_start(wv[:], wv_in[:])
                nc.sync.dma_start(masks[:], masks_in[:])
                nc.sync.dma_start(bias_sb[:], bias_in[:])
                nc.sync.dma_start(bqp[:], bqp_in[:])
                nc.sync.dma_start(wp[:], wp_in[:])

                # ~3us of junk DoubleRow matmuls on the ones tile: ramps the
                # PE p-state while the first x/w DMAs are still in flight
                wps = pp.tile([P, 512], FP, tag="ps", name="wps")
                for _ in range(60):
                    nc.tensor.matmul(
                        wps[:, 0:P],
                        lhsT=ones_sb[:],
                        rhs=ones_sb[:, :, :],
                        start=True,
                        stop=True,
                        perf_mode=DR,
                        skip_group_check=True,
                    )

                xhalf = [xo, xx]

                def mm12(ps, w, x, osl, cs, ce, n_start=True, n_stop=True):
                    """12-op compensated group: out[osl, cs:ce] += w.T @ x.
                    w stored (lo,hi), x stored (hi,lo); contraction over all
                    NT c-subtiles."""
                    first = [n_start]
                    for t in range(NT // 2):
                        nc.tensor.matmul(
                            ps[:, : ce - cs],
                            lhsT=w[:, 1, 2 * t : 2 * t + 2, osl],
                            rhs=x[:, 0, 2 * t : 2 * t + 2, cs:ce],
                            start=first[0],
                            stop=False,
                            perf_mode=DR,
                        )
                        first[0] = False
                    for ct in range(NT):
                        nc.tensor.matmul(
                            ps[:, : ce - cs],
                            lhsT=w[:, 0:2, ct, osl],
                            rhs=x[:, 0:2, ct, cs:ce],
                            start=False,
                            stop=(n_stop and ct == NT - 1),
                            perf_mode=DR,
                        )

                # ---- k projection (no bias; softmax-invariant) ----------
                sc = tc.nc.named_scope("A_k"); sc.__enter__()
                for half, cs in ((0, 0), (0, 512), (1, 0), (1, 512)):
                    xh = xhalf[half]
                    ce = cs + 512
                    for ot in range(NT):
                        osl = slice(ot * P, (ot + 1) * P)
                        ps = pp.tile([P, 512], FP, tag="ps")
                        mm12(ps, wk, xh, osl, cs, ce)
                        g0 = half * H + cs
                        nc.scalar.activation(
                            kT[:, 1, ot, g0 : g0 + 512], ps[:], AF.Identity
                        )
                        nc.vector.tensor_sub(
                            kT[:, 0, ot, g0 : g0 + 512],
                            ps[:],
                            kT[:, 1, ot, g0 : g0 + 512],
                        )
                sc.__exit__(None, None, None)

                # ---- bqk: per-kv-position q-bias term (bq~ . k_j) -------
                sc = tc.nc.named_scope("A_bqk"); sc.__enter__()
                psb_pool = tc.tile_pool(name="psb", bufs=1, space="PSUM")
                ppb = psb_pool.__enter__()
                ps_b = ppb.tile([P, NKV], FP, tag="psb")
                nop = 0
                for kvt in range(NKV):
                    ksl = slice(kvt * P, (kvt + 1) * P)
                    for t in range(NT // 2):
                        nc.tensor.matmul(
                            ps_b[:, kvt : kvt + 1],
                            lhsT=kT[:, 1, 2 * t : 2 * t + 2, ksl],
                            rhs=bqp[:, 0, 2 * t : 2 * t + 2, :],
                            start=(nop == 0),
                            stop=False,
                            perf_mode=DR,
                            skip_group_check=True,
                        )
                        nop += 1
                    for ct in range(NT):
                        nop += 1
                        nc.tensor.matmul(
                            ps_b[:, kvt : kvt + 1],
                            lhsT=kT[:, 0:2, ct, ksl],
                            rhs=bqp[:, 0:2, ct, :],
                            start=False,
                            stop=(nop == 12 * NKV),
                            perf_mode=DR,
                            skip_group_check=True,
                        )
                # bqk_sb = bqk - ln(ASCALE): the Exp bias for each kv row
                nc.scalar.activation(
                    bqk_sb[:], ps_b[:], AF.Identity, bias=bias_sb[:, NT : NT + 1]
                )
                psb_pool.__exit__(None, None, None)
                sc.__exit__(None, None, None)

                # ---- v projection (x stationary, w moving; no bias) -----
                sc = tc.nc.named_scope("A_v"); sc.__enter__()
                for half in range(2):
                    xh = xhalf[half]
                    for tt in range(NT):
                        ts_g = half * NT + tt
                        tsl = slice(tt * P, (tt + 1) * P)
                        for cs, ce in ((0, 512), (512, 1024)):
                            ps = pp.tile([P, 512], FP, tag="ps")
                            first = True
                            for t in range(NT // 2):
                                nc.tensor.matmul(
                                    ps[:],
                                    lhsT=xh[:, 0, 2 * t : 2 * t + 2, tsl],
                                    rhs=wv[:, 1, 2 * t : 2 * t + 2, cs:ce],
                                    start=first,
                                    stop=False,
                                    perf_mode=DR,
                                )
                                first = False
                            for ct in range(NT):
                                nc.tensor.matmul(
                                    ps[:],
                                    lhsT=xh[:, 0:2, ct, tsl],
                                    rhs=wv[:, 0:2, ct, cs:ce],
                                    start=False,
                                    stop=(ct == NT - 1),
                                    perf_mode=DR,
                                )
                            nc.scalar.activation(
                                V[:, 1, ts_g, cs:ce], ps[:], AF.Identity
                            )
                            nc.vector.tensor_sub(
                                V[:, 0, ts_g, cs:ce], ps[:], V[:, 1, ts_g, cs:ce]
                            )
                sc.__exit__(None, None, None)

                # ---- q projection (scaled wq; bias via bqk) -------------
                sc = tc.nc.named_scope("A_q"); sc.__enter__()
                for ot in range(NT):
                    osl = slice(ot * P, (ot + 1) * P)
                    for cs, ce in ((0, 512), (512, 1024)):
                        ps = pp.tile([P, 512], FP, tag="ps")
                        mm12(ps, wq, xq, osl, cs, ce)
                        nc.scalar.activation(
                            qT[:, 0, ot, cs:ce], ps[:], AF.Identity
                        )
                        nc.vector.tensor_sub(
                            qT[:, 1, ot, cs:ce], ps[:], qT[:, 0, ot, cs:ce]
                        )
                sc.__exit__(None, None, None)

            # =============================================================
            # Phases B-D (attention): xw freed; AT/Oacc/Opair reuse space
            # =============================================================
            with (
                tc.tile_pool(name="attn", bufs=1) as ab,
                tc.tile_pool(name="efp", bufs=4) as efp,
            ):
                AT = ab.tile([P, 2, NKV, H], F8, tag="AT")   # (hi, lo)
                Oacc = ab.tile([P, NT, H], FP, tag="Oacc")
                Opair = ab.tile([P, 2, NT, H], F8, tag="Op")  # (hi, lo)
                rs_sb = ab.tile([P, H], FP, tag="rs_sb")
                rs_ps = pp_rs.tile([P, H], FP, tag="rs")

                ef_cur = [None]

                def scores_s(s):
                    lo_s = LO16[s] * P
                    lo_p = LOP16[s] * P
                    if s % 2 == 0:
                        ef_cur[0] = efp.tile([P, 2, H], BF, tag="ef", name=f"ef{s}")
                    ef = ef_cur[0]
                    chs = _chunks512(lo_p, H)
                    pss = [
                        pp.tile([P, ce - cs], FP, tag="ps", name=f"pss{s}_{cs}")
                        for cs, ce in chs
                    ]
                    # ct-outer so each stationary kT slice loads once
                    nop, last = 0, 12 * len(chs)
                    for t in range(NT // 2):
                        for ps, (cs, ce) in zip(pss, chs):
                            mlo = max(cs, lo_s)
                            nc.tensor.matmul(
                                ps[:, mlo - cs : ce - cs],
                                lhsT=kT[:, 1, 2 * t : 2 * t + 2, s * P : (s + 1) * P],
                                rhs=qT[:, 0, 2 * t : 2 * t + 2, mlo:ce],
                                start=(nop < len(chs)),
                                stop=False,
                                perf_mode=DR,
                                skip_group_check=True,
                            )
                            nop += 1
                    for ct in range(NT):
                        for ps, (cs, ce) in zip(pss, chs):
                            mlo = max(cs, lo_s)
                            nop += 1
                            nc.tensor.matmul(
                                ps[:, mlo - cs : ce - cs],
                                lhsT=kT[:, 0:2, ct, s * P : (s + 1) * P],
                                rhs=qT[:, 0:2, ct, mlo:ce],
                                start=False,
                                stop=(nop > last - len(chs)),
                                perf_mode=DR,
                                skip_group_check=True,
                            )
                    # dead sliver [lo_p, lo_s): exp(-1e9) = 0 keeps the fp8
                    # pair exactly zero there so paired attv ops read zeros
                    if lo_s > lo_p:
                        nc.vector.memset(pss[0][:, 0 : lo_s - lo_p], NEG)
                    for ps, (cs, ce) in zip(pss, chs):
                        for blk, mi in MASKS16[s]:
                            a = blk * P
                            if cs <= a < ce:
                                nc.vector.tensor_add(
                                    ps[:, a - cs : a - cs + P],
                                    ps[:, a - cs : a - cs + P],
                                    masks[:, mi, :],
                                )
                        nc.scalar.activation(
                            ef[:, s % 2, cs:ce],
                            ps[:],
                            AF.Exp,
                            bias=bqk_sb[:, s : s + 1],
                        )
                    if s % 2 == 1:
                        # pair complete: decompose exp into the AT fp8 pair
                        # (hi copy on Pool, lo sub on DVE -- splits the ~28us
                        # of decompose across the two engines with slack)
                        nc.gpsimd.tensor_copy(
                            AT[:, 0, s - 1 : s + 1, lo_p:H], ef[:, :, lo_p:H]
                        )
                        nc.vector.tensor_sub(
                            AT[:, 1, s - 1 : s + 1, lo_p:H],
                            ef[:, :, lo_p:H],
                            AT[:, 0, s - 1 : s + 1, lo_p:H],
                        )
                        # rowsum for both subtiles of the pair
                        for sj in (s - 1, s):
                            lo_j = LOP16[sj] * P
                            for cs, ce in _chunks512(lo_j, H):
                                nc.tensor.matmul(
                                    rs_ps[:, cs:ce],
                                    lhsT=ones_sb[:],
                                    rhs=AT[:, 0:2, sj, cs:ce],
                                    start=(sj == 0),
                                    stop=(
                                        (cs < 512 and sj == 7)
                                        or (cs >= 512 and sj == NKV - 1)
                                    ),
                                    perf_mode=DR,
                                    skip_group_check=True,
                                )

                def attv_chunk(ci, pairs):
                    cs, ce = ci * 512, (ci + 1) * 512
                    for ot in range(NT):
                        osl = slice(ot * P, (ot + 1) * P)
                        ps = pp.tile([P, 512], FP, tag="ps")
                        nops = 3 * len(pairs)
                        nop = 0
                        for p in pairs:
                            plo = max(cs, LOP16[2 * p] * P)
                            nc.tensor.matmul(
                                ps[:, plo - cs : 512],
                                lhsT=V[:, 1, 2 * p : 2 * p + 2, osl],
                                rhs=AT[:, 0, 2 * p : 2 * p + 2, plo:ce],
                                start=(nop == 0),
                                stop=False,
                                perf_mode=DR,
                                skip_group_check=True,
                            )
                            nop += 1
                            for sj in (2 * p, 2 * p + 1):
                                nop += 1
                                nc.tensor.matmul(
                                    ps[:, plo - cs : 512],
                                    lhsT=V[:, 0:2, sj, osl],
                                    rhs=AT[:, 0:2, sj, plo:ce],
                                    start=False,
                                    stop=(nop == nops),
                                    perf_mode=DR,
                                    skip_group_check=True,
                                )
                        nc.vector.tensor_copy(Oacc[:, ot, cs:ce], ps[:])

                def normalize_chunk(ci):
                    cs, ce = ci * 512, (ci + 1) * 512
                    nc.vector.tensor_copy(rs_sb[:, cs:ce], rs_ps[:, cs:ce])
                    nc.vector.reciprocal(rs_sb[:, cs:ce], rs_sb[:, cs:ce])
                    for ot in range(NT):
                        nc.vector.tensor_mul(
                            Oacc[:, ot, cs:ce], Oacc[:, ot, cs:ce], rs_sb[:, cs:ce]
                        )
                        nc.scalar.activation(
                            Opair[:, 0, ot, cs:ce], Oacc[:, ot, cs:ce], AF.Identity
                        )
                        nc.vector.tensor_sub(
                            Opair[:, 1, ot, cs:ce],
                            Oacc[:, ot, cs:ce],
                            Opair[:, 0, ot, cs:ce],
                        )

                def oproj_chunk(ci, evac):
                    cs, ce = ci * 512, (ci + 1) * 512
                    for o2 in range(NT):
                        osl = slice(o2 * P, (o2 + 1) * P)
                        ps = pp.tile([P, 512], FP, tag="ps")
                        first = True
                        for t in range(NT // 2):
                            nc.tensor.matmul(
                                ps[:],
                                lhsT=wp[:, 1, 2 * t : 2 * t + 2, osl],
                                rhs=Opair[:, 0, 2 * t : 2 * t + 2, cs:ce],
                                start=first,
                                stop=False,
                                perf_mode=DR,
                            )
                            first = False
                        for ct in range(NT):
                            nc.tensor.matmul(
                                ps[:],
                                lhsT=wp[:, 0:2, ct, osl],
                                rhs=Opair[:, 0:2, ct, cs:ce],
                                start=False,
                                stop=(ct == NT - 1),
                                perf_mode=DR,
                            )
                        ev = evac.tile([P, 512], BF, tag="evy", name=f"evy{ci}_{o2}")
                        nc.scalar.activation(
                            ev[:], ps[:], AF.Identity, bias=bias_sb[:, o2 : o2 + 1]
                        )
                        nc.sync.dma_start(
                            yT[(o2 * 2 + ci) * P : (o2 * 2 + ci + 1) * P, :],
                            ev[:],
                        )

                sc = tc.nc.named_scope("B1"); sc.__enter__()
                for s in range(16):
                    scores_s(s)
                sc.__exit__(None, None, None)
                with tc.tile_pool(name="evac", bufs=3) as evac:
                    sc = tc.nc.named_scope("B2"); sc.__enter__()
                    attv_chunk(0, [0, 1, 2, 3])
                    normalize_chunk(0)
                    sc.__exit__(None, None, None)
                    sc = tc.nc.named_scope("D2"); sc.__enter__()
                    attv_chunk(1, [0, 1, 2, 3, 4, 5, 6, 7])
                    normalize_chunk(1)
                    sc.__exit__(None, None, None)
                    sc = tc.nc.named_scope("E"); sc.__enter__()
                    oproj_chunk(0, evac)
                    oproj_chunk(1, evac)
                    sc.__exit__(None, None, None)

    _split_waits(nc)
    return nc


_NC_CACHE = None


def _get_nc():
    global _NC_CACHE
    if _NC_CACHE is None:
        _NC_CACHE = _build_nc()
    return _NC_CACHE


def _pair(a, order="hl"):
    """Decompose fp32 array -> fp8 (hi, lo) or (lo, hi) pair along new axis 1.
    a: [P, ...]; returns [P, 2, ...] float8_e4m3."""
    import ml_dtypes

    a = np.asarray(a, dtype=np.float32)
    hi = a.astype(ml_dtypes.float8_e4m3)
    lo = (a - hi.astype(np.float32)).astype(ml_dtypes.float8_e4m3)
    pair = (hi, lo) if order == "hl" else (lo, hi)
    return np.ascontiguousarray(np.stack(pair, axis=1))


def _tile_major(m):
    """[C_in, N] -> [P, C_in//P, N] with partition dim first."""
    cin, n = m.shape
    return np.ascontiguousarray(m.reshape(cin // P, P, n).transpose(1, 0, 2))


def make_in_maps(x, w_qkv, b_qkv, w_proj, b_proj):
    """Host-side prep: shard + transpose + fp8-decompose inputs for 8 cores."""
    x = np.asarray(x, dtype=np.float32)
    w_qkv = np.asarray(w_qkv, dtype=np.float32)
    b_qkv = np.asarray(b_qkv, dtype=np.float32)
    w_proj = np.asarray(w_proj, dtype=np.float32)
    b_proj = np.asarray(b_proj, dtype=np.float32)
    import ml_dtypes

    s = 1.0 / np.sqrt(np.float32(C))

    # weights, stored (lo, hi), layout [P cpart, 2, NT csub, C out]
    wq = _pair(_tile_major((w_qkv[0:C] * s).T), "lh")
    wk = _pair(_tile_major(w_qkv[C : 2 * C].T), "lh")
    wv = _pair(_tile_major(w_qkv[2 * C : 3 * C].T), "lh")
    wp = _pair(_tile_major(w_proj.T), "lh")
    bqp = _pair(_tile_major((b_qkv[0:C] * s).reshape(C, 1)), "hl")

    bv = b_qkv[2 * C : 3 * C]
    beff = (b_proj + w_proj @ bv).reshape(NT, P).T
    bias = np.concatenate(
        [beff, np.full((P, 1), -np.log(ASCALE), np.float32)], axis=1
    ).astype(np.float32)

    ones = np.ones((P, 2, P), dtype=np.float32).astype(ml_dtypes.float8_e4m3)

    # S^T mask tiles: partition = kv j (within subtile), free = query i
    triu = np.triu(np.ones((P, P), dtype=np.float32))
    trilm = np.where(triu > 0, 0.0, NEG).astype(np.float32)
    zeros = np.zeros((P, P), dtype=np.float32)
    negs = np.full((P, P), NEG, dtype=np.float32)

    shared = dict(
        wq_in=wq, wk_in=wk, wv_in=wv, wp_in=wp, bqp_in=bqp,
        bias_in=bias, ones_in=ones,
    )
    in_maps = []
    for core in range(8):
        b, h = core // 2, core % 2
        xb = x[b]  # [T, C]
        qrows = np.concatenate(
            [xb[(2 * bg + h) * 256 : (2 * bg + h + 1) * 256] for bg in range(4)],
            axis=0,
        )
        m = np.stack(
            [
                trilm if h == 0 else zeros,   # m1d
                negs if h == 0 else zeros,    # m1f
                negs if h == 0 else trilm,    # m2d
            ],
            axis=1,
        )
        in_maps.append(
            dict(
                shared,
                xq_in=_pair(_tile_major(qrows.T), "hl"),
                xo_in=_pair(_tile_major(xb[0:H].T), "hl"),
                xx_in=_pair(_tile_major(xb[H : 2 * H].T), "hl"),
                masks_in=np.ascontiguousarray(m),
            )
        )
    return in_maps


def assemble_output(results):
    B = 4
    y = np.empty((B, 2 * H, C), dtype=np.float32)
    for core in range(8):
        b, h = core // 2, core % 2
        yt = results[core]["yT"].astype(np.float32).reshape(NT, 2, P, 512)
        blk = yt.transpose(1, 3, 0, 2).reshape(H, C)
        blk4 = blk.reshape(4, 256, C)
        for bg in range(4):
            g = 2 * bg + h
            y[b, g * 256 : (g + 1) * 256, :] = blk4[bg]
    return y


def kernel(x, w_qkv, b_qkv, w_proj, b_proj):
    from concourse.bass_utils import run_bass_kernel_spmd

    nc = _get_nc()
    in_maps = make_in_maps(x, w_qkv, b_qkv, w_proj, b_proj)
    res = run_bass_kernel_spmd(nc, in_maps, list(range(8)))
    return assemble_output(res.results)
